# revision 1
# baseline (speedup 1.0000x reference)
"""Trainium2 Bass kernel for nn_Attention (cumulative masked softmax attention).

Reference computation:
    v   = tanh(x @ W + b)                  (B, T, F)
    a   = v . u                            (B, T)   -- query-independent logits
    e   = exp(a)[:, None, :] * tril * mask (B, T, T)
    alf = e / (sum_s e + EPS)
    c   = alf @ x                          (B, T, F)

Because the logits are query-independent and the mask is lower-triangular,
the (B,T,T) softmax-matmul collapses to a running weighted average:
    w[s]  = exp(a[s]) * mask[s]
    c[t]  = cumsum_s(w * x)[t] / (cumsum_s(w)[t] + EPS)
which is O(B*T*F) instead of O(B*T^2*F).

Sharding: data-parallel over batch B across 8 NeuronCores (2 batches/core).
W/u/b replicated. Each core processes 2048 rows of (T, F); the (w*x) cumsum
is done per-batch with triangular/ones matmul blocks on the tensor engine,
and the scalar cumsum of w via two tiny matmuls + a free-dim prefix scan.
Matmul operands use float32r (fp32, ~11-bit mantissa, full PE rate).
The host supplies x both in natural layout and pre-transposed (xT) so the
tensor engine needs no on-chip transposes for the x @ W contraction.
"""

import numpy as np

import concourse.bass as bass  # noqa: F401
import concourse.tile as tile
from concourse import bacc, mybir
from concourse.bass_utils import run_bass_kernel_spmd

B, T, F = 16, 1024, 512
EPS = 1e-7
NCORES = 8
B_LOC = B // NCORES          # batches per core
R = B_LOC * T                # rows per core
P = 128                      # partition tile
NT = R // P                  # row tiles per core
NTB = T // P                 # row tiles per batch
KC = F // P                  # contraction chunks

F32 = mybir.dt.float32
F32R = mybir.dt.float32r


def _build(have_b: bool, have_mask: bool, loop_n: int = 0):
    """Build the per-core Bass module. loop_n > 0 wraps the body in a
    hardware For_i loop (used only for timing)."""
    nc = bacc.Bacc("TRN2", target_bir_lowering=False, debug=False)

    x_d = nc.dram_tensor("x", [NT, P, F], F32, kind="ExternalInput")
    xt_d = nc.dram_tensor("xT", [NT, P, F], F32R, kind="ExternalInput")
    # W pre-arranged on host as (P, KC*F): W_host[p, k*F+f] = W[k*P+p, f]
    w_d = nc.dram_tensor("W", [P, KC * F], F32R, kind="ExternalInput")
    u_d = nc.dram_tensor("u", [1, F], F32, kind="ExternalInput")
    if have_b:
        b_d = nc.dram_tensor("b", [1, F], F32, kind="ExternalInput")
    if have_mask:
        m_d = nc.dram_tensor("m", [NT, P, 1], F32, kind="ExternalInput")
    c_d = nc.dram_tensor("c", [NT, P, F], F32, kind="ExternalOutput")

    Tanh = mybir.ActivationFunctionType.Tanh
    Exp = mybir.ActivationFunctionType.Exp
    Copy = mybir.ActivationFunctionType.Copy
    ADD = mybir.AluOpType.add
    SUB = mybir.AluOpType.subtract

    NP = NT // 2             # tile pairs

    with tile.TileContext(nc) as tc:
        with (
            tc.tile_pool(name="const", bufs=1) as const,
            tc.tile_pool(name="xp", bufs=6) as xp,
            tc.tile_pool(name="xtp", bufs=6) as xtp,
            tc.tile_pool(name="vp", bufs=2) as vp,
            tc.tile_pool(name="scrp", bufs=2) as scrp,
            tc.tile_pool(name="yp", bufs=NT) as yp,
            tc.tile_pool(name="wap", bufs=B_LOC) as wap,
            tc.tile_pool(name="smal", bufs=6) as smal,
            tc.tile_pool(name="cp", bufs=3) as cp,
            tc.tile_pool(name="ps_v", bufs=2, space="PSUM") as ps_v_pool,
            tc.tile_pool(name="ps_P", bufs=2, space="PSUM") as ps_P_pool,
            tc.tile_pool(name="ps_Z", bufs=1, space="PSUM") as ps_Z_pool,
        ):
            # ---- constants ----
            W_sb = const.tile([P, KC, F], F32R)
            nc.sync.dma_start(out=W_sb,
                              in_=w_d.ap().rearrange("p (k f) -> p k f", k=KC))
            u_bc2 = const.tile([P, 2, F], F32)
            nc.gpsimd.dma_start(out=u_bc2[:, 0, :],
                                in_=u_d.ap().to_broadcast((P, F)))
            nc.gpsimd.dma_start(out=u_bc2[:, 1, :],
                                in_=u_d.ap().to_broadcast((P, F)))
            if have_b:
                b_sb = const.tile([1, F], F32R)
                bf = smal.tile([1, F], F32, tag="bstage")
                nc.sync.dma_start(out=bf, in_=b_d.ap())
                nc.vector.tensor_copy(b_sb, bf)
                ones_row = const.tile([1, P], F32R)
                nc.vector.memset(ones_row, 1.0)
            # triangular + ones matmul weights for the cumsum (exact in f32r)
            triu_f = const.tile([P, P], F32)
            nc.gpsimd.memset(triu_f, 0.0)
            nc.gpsimd.affine_select(
                out=triu_f, in_=triu_f, compare_op=mybir.AluOpType.is_gt,
                fill=1.0, base=0, pattern=[[-1, P]], channel_multiplier=1)
            triu = const.tile([P, P], F32R)
            nc.vector.tensor_copy(triu, triu_f)
            ones = const.tile([P, P], F32R)
            onesf = const.tile([P, P], F32)
            nc.vector.memset(onesf, 1.0)
            nc.vector.tensor_copy(ones, onesf)
            zeros8 = const.tile([P, NTB], F32)
            nc.vector.memset(zeros8, 0.0)

            import contextlib
            loop_ctx = (tc.For_i(0, loop_n, 1) if loop_n
                        else contextlib.nullcontext())
            with loop_ctx:
                ys = []
                w_alls = []
                # ---- phase A: logits -> weights w, weighted values y ----
                for pp in range(NP):
                    i0 = 2 * pp
                    if i0 % NTB == 0:
                        w_all = wap.tile([P, NTB], F32)
                        w_alls.append(w_all)

                    ps_v2 = ps_v_pool.tile([P, 2, F], F32)
                    ldeng = nc.sync
                    xT2 = xtp.tile([P, 2, F], F32R)
                    ldeng.dma_start(
                        out=xT2,
                        in_=xt_d.ap()[i0:i0 + 2].rearrange("j p f -> p j f"))
                    xt2 = xp.tile([P, 2, F], F32)
                    ldeng.dma_start(
                        out=xt2,
                        in_=x_d.ap()[i0:i0 + 2].rearrange("j p f -> p j f"))
                    for j in range(2):
                        for k in range(KC):
                            nc.tensor.matmul(
                                ps_v2[:, j, :],
                                xT2[:, j, k * P:(k + 1) * P],
                                W_sb[:, k, :],
                                start=(k == 0),
                                stop=(k == KC - 1 and not have_b),
                            )
                        if have_b:
                            nc.tensor.matmul(ps_v2[:, j, :], ones_row, b_sb,
                                             start=False, stop=True)
                    xts = [xt2[:, 0, :], xt2[:, 1, :]]

                    v2 = vp.tile([P, 2, F], F32)
                    nc.scalar.activation(out=v2, in_=ps_v2, func=Tanh)
                    scr2 = scrp.tile([P, 2, F], F32)
                    nc.vector.tensor_mul(scr2, v2, u_bc2)
                    alpha2 = smal.tile([P, 2], F32)
                    nc.vector.tensor_reduce(alpha2, scr2,
                                            axis=mybir.AxisListType.X, op=ADD)
                    ib0 = i0 % NTB
                    nc.scalar.activation(out=w_all[:, ib0:ib0 + 2],
                                         in_=alpha2, func=Exp)
                    if have_mask:
                        mt = smal.tile([P, 2], F32)
                        for j in range(2):
                            nc.sync.dma_start(out=mt[:, j:j + 1],
                                              in_=m_d.ap()[i0 + j])
                        nc.vector.tensor_mul(w_all[:, ib0:ib0 + 2],
                                             w_all[:, ib0:ib0 + 2], mt)
                    for j in range(2):
                        y = yp.tile([P, F], F32R)
                        nc.gpsimd.tensor_scalar_mul(
                            y, xts[j], w_all[:, ib0 + j:ib0 + j + 1])
                        ys.append(y)

                # ---- phase B: Z prefixes then blockwise cumsum ----
                recs = []
                for batch in range(B_LOC):
                    wr_all = wap.tile([P, NTB], F32R, tag="wr")
                    nc.vector.tensor_copy(wr_all, w_alls[batch])
                    ps_A = ps_Z_pool.tile([P, NTB], F32)
                    ps_B = ps_Z_pool.tile([P, NTB], F32)
                    nc.tensor.matmul(ps_A, triu, wr_all, start=True, stop=True)
                    nc.tensor.matmul(ps_B, ones, wr_all, start=True, stop=True)
                    S = smal.tile([P, NTB], F32)
                    nc.vector.tensor_tensor_scan(
                        out=S, data0=ps_B, data1=zeros8, initial=0.0,
                        op0=ADD, op1=ADD)
                    D = smal.tile([P, NTB], F32)
                    nc.vector.tensor_tensor(out=D, in0=S, in1=ps_B, op=SUB)
                    Z = smal.tile([P, NTB], F32)
                    nc.vector.tensor_tensor(out=Z, in0=D, in1=ps_A, op=ADD)
                    zr = smal.tile([P, NTB], F32)
                    nc.vector.tensor_scalar_add(zr, Z, EPS)
                    rec = smal.tile([P, NTB], F32, tag="rec")
                    nc.vector.reciprocal(rec, zr)
                    recs.append(rec)

                # second-level pair sums u_q = y[2q] + y[2q+1]
                us = []
                for q in range(NT // 2):
                    upair = yp.tile([P, F], F32R, tag="us")
                    nc.vector.tensor_tensor(out=upair, in0=ys[2 * q],
                                            in1=ys[2 * q + 1], op=ADD)
                    us.append(upair)
                cs = []
                for i in range(NT):
                    ib = i % NTB
                    base = i - ib
                    batch = i // NTB
                    mms = [(triu, ys[i])]
                    if ib % 2 == 1:
                        mms.append((ones, ys[i - 1]))
                    mms += [(ones, us[base // 2 + q]) for q in range(ib // 2)]
                    ps_P = ps_P_pool.tile([P, F], F32)
                    for n_, (lh, rh) in enumerate(mms):
                        nc.tensor.matmul(ps_P, lh, rh, start=(n_ == 0),
                                         stop=(n_ == len(mms) - 1))
                    if i % 2 == 0:
                        c2 = cp.tile([P, 2, F], F32)
                        cs.append(c2)
                        nc.scalar.activation(out=c2[:, 0, :], in_=ps_P,
                                             func=Copy,
                                             scale=recs[batch][:, ib:ib + 1])
                    else:
                        nc.vector.tensor_scalar_mul(
                            cs[-1][:, 1, :], ps_P, recs[batch][:, ib:ib + 1])
                        nc.scalar.dma_start(
                            out=c_d.ap()[i - 1:i + 1].rearrange(
                                "j p f -> p j f"),
                            in_=cs[-1])

    nc.compile()
    return nc


_NC_CACHE: dict = {}


def _get_nc(have_b, have_mask, loop_n=0):
    key = (have_b, have_mask, loop_n)
    if key not in _NC_CACHE:
        _NC_CACHE[key] = _build(have_b, have_mask, loop_n)
    return _NC_CACHE[key]


def _host_xt(xs):
    """xs: (NT, P, F) tile-major core shard -> pre-transposed layout where
    xt[i, p, k*128+t] = xs[i, t, k*128+p] (chunk-transposed for matmul lhsT)."""
    v = xs.reshape(NT, P, KC, P).transpose(0, 3, 2, 1)
    return np.ascontiguousarray(v).reshape(NT, P, F)


def make_core_maps(x, W, u, b=None, mask_f=None):
    """Build the 8 per-core input maps from full inputs."""
    # W_host[p, k*F + f] = W[k*P + p, f]
    W_r = np.ascontiguousarray(
        W.reshape(KC, P, F).transpose(1, 0, 2).reshape(P, KC * F))
    u_r = np.ascontiguousarray(u.reshape(1, F))
    maps = []
    for core in range(NCORES):
        xs = np.ascontiguousarray(
            x[core * B_LOC:(core + 1) * B_LOC].reshape(NT, P, F))
        m = {"x": xs, "xT": _host_xt(xs), "W": W_r, "u": u_r}
        if b is not None:
            m["b"] = np.ascontiguousarray(b.reshape(1, F))
        if mask_f is not None:
            m["m"] = np.ascontiguousarray(
                mask_f[core * B_LOC:(core + 1) * B_LOC].reshape(NT, P, 1))
        maps.append(m)
    return maps


def kernel(x, mask, W, b, u):
    x = np.asarray(x, dtype=np.float32)
    W = np.asarray(W, dtype=np.float32)
    b = np.asarray(b, dtype=np.float32)
    u = np.asarray(u, dtype=np.float32)
    mask_f = np.asarray(mask).astype(np.float32)

    have_b = bool(np.any(b != 0.0))
    have_mask = bool(np.any(mask_f != 1.0))

    nc = _get_nc(have_b, have_mask)
    in_maps = make_core_maps(x, W, u,
                             b if have_b else None,
                             mask_f if have_mask else None)
    res = run_bass_kernel_spmd(nc, in_maps, core_ids=list(range(NCORES)))
    out = np.stack([r["c"].reshape(B_LOC, T, F) for r in res.results])
    return out.reshape(B, T, F)



# revision 36
# speedup vs baseline: 2.0709x; 2.0709x over previous
"""Trainium2 Bass kernel for nn_Attention (cumulative masked softmax attention).

Reference computation:
    v   = tanh(x @ W + b)                  (B, T, F)
    a   = v . u                            (B, T)   -- query-independent logits
    e   = exp(a)[:, None, :] * tril * mask (B, T, T)
    alf = e / (sum_s e + EPS)
    c   = alf @ x                          (B, T, F)

Because the logits are query-independent and the mask is lower-triangular,
the (B,T,T) softmax-matmul collapses to a running weighted average:
    w[s]  = exp(a[s]) * mask[s]
    c[t]  = cumsum_s(w * x)[t] / (cumsum_s(w)[t] + EPS)
which is O(B*T*F) instead of O(B*T^2*F).

Sharding: data-parallel over batch B across 8 NeuronCores (2 batches/core).

v3 design (bf16):
  - All HBM traffic in bf16 (x, xT, W, u, c out) -- rel-err budget is 2e-2,
    bf16 keeps it ~2e-3 while halving DMA bytes.
  - The weights w fold into the 128x128 triangular matrix (tri_w = tri * w
    per tile, a cheap [128,128] gpsimd scale) instead of scaling x.
  - Cross-tile prefix offsets via 7 "step-mask" matmuls per batch (lhsT
    column m gets w[:,j] iff m > j) writing the per-tile offset rows [8, F]
    in PSUM; one copy to SBUF, then one [8,128]-selector matmul per tile
    broadcasts its offset row onto the tile prefix.
  - Denominator Z = prefix(w) via two tiny [P,8] matmuls + a free-dim scan
    (f32), reciprocal folded into the PSUM->SBUF readout scale.
  - Scheduling: xT loads split across the SP and ACT DMA queues ahead of
    the x loads; exp/tri-scales issued per pair so phase C of batch 0 can
    fill the tensor engine while batch 1's logits chain completes.
"""

import contextlib

import numpy as np
import ml_dtypes

import concourse.bass as bass  # noqa: F401
import concourse.tile as tile
from concourse import bacc, mybir
from concourse.bass_utils import run_bass_kernel_spmd

B, T, F = 16, 1024, 512
EPS = 1e-7
NCORES = 8
B_LOC = B // NCORES          # batches per core
R = B_LOC * T                # rows per core
P = 128                      # partition tile
NT = R // P                  # row tiles per core
NTB = T // P                 # row tiles per batch
NPAIR = NTB // 2             # tile pairs per batch
KC = F // P                  # contraction chunks

F32 = mybir.dt.float32
F32R = mybir.dt.float32r
BF16 = mybir.dt.bfloat16
NPBF16 = ml_dtypes.bfloat16


def _build(have_b: bool, have_mask: bool, loop_n: int = 0):
    """Build the per-core Bass module. loop_n > 0 wraps the body in a
    hardware For_i loop (used only for timing)."""
    nc = bacc.Bacc("TRN2", target_bir_lowering=False, debug=False)

    x_d = nc.dram_tensor("x", [NT, P, F], BF16, kind="ExternalInput")
    xt_d = nc.dram_tensor("xT", [NT, P, F], BF16, kind="ExternalInput")
    # W pre-arranged on host as (P, KC*F): W_host[p, k*F+f] = W[k*P+p, f]
    w_d = nc.dram_tensor("W", [P, KC * F], BF16, kind="ExternalInput")
    u_d = nc.dram_tensor("u", [1, F], BF16, kind="ExternalInput")
    if have_b:
        b_d = nc.dram_tensor("b", [1, F], BF16, kind="ExternalInput")
    if have_mask:
        m_d = nc.dram_tensor("m", [B_LOC, P, NTB], F32, kind="ExternalInput")
    c_d = nc.dram_tensor("c", [NT, P, F], BF16, kind="ExternalOutput")

    Tanh = mybir.ActivationFunctionType.Tanh
    Exp = mybir.ActivationFunctionType.Exp
    Copy = mybir.ActivationFunctionType.Copy
    ADD = mybir.AluOpType.add
    SUB = mybir.AluOpType.subtract

    with tile.TileContext(nc) as tc:
        with (
            tc.tile_pool(name="const", bufs=1) as const,
            tc.tile_pool(name="xp", bufs=2 * NPAIR) as xp,
            tc.tile_pool(name="xtp", bufs=2 * NPAIR) as xtp,
            tc.tile_pool(name="vp", bufs=2) as vp,
            tc.tile_pool(name="scrp", bufs=2) as scrp,
            tc.tile_pool(name="foldp", bufs=2) as foldp,
            tc.tile_pool(name="wp", bufs=2) as wp,
            tc.tile_pool(name="trwp", bufs=10) as trwp,
            tc.tile_pool(name="lwp", bufs=8) as lwp,
            tc.tile_pool(name="cumbp", bufs=2) as cumbp,
            tc.tile_pool(name="cp", bufs=6) as cp,
            tc.tile_pool(name="ps_v", bufs=2, space="PSUM") as ps_v_pool,
            tc.tile_pool(name="ps_P", bufs=4, space="PSUM") as ps_P_pool,
        ):
            # ---- constants ----
            # W split into per-chunk DMAs on the gpsimd queue so the first
            # matmul only waits on chunk 0 (+ the first xT tile) and the SP
            # queue can issue the xT loads immediately.
            W_sb = const.tile([P, KC, F], BF16)
            wr_ap = w_d.ap().rearrange("p (k f) -> p k f", k=KC)
            for k in range(KC):
                nc.gpsimd.dma_start(out=W_sb[:, k, :], in_=wr_ap[:, k, :])
            u_bc2 = const.tile([P, 2, F], BF16)
            nc.gpsimd.dma_start(out=u_bc2[:, 0, :],
                                in_=u_d.ap().to_broadcast((P, F)))
            nc.gpsimd.dma_start(out=u_bc2[:, 1, :],
                                in_=u_d.ap().to_broadcast((P, F)))
            if have_b:
                b_sb = const.tile([1, F], BF16)
                nc.sync.dma_start(out=b_sb, in_=b_d.ap())
                ones_row = const.tile([1, P], BF16)
                nc.vector.memset(ones_row, 1.0)

            # triangular matrices: triu[p, m] = 1 iff p <= m (inclusive prefix)
            triu_f = const.tile([P, P], F32)
            nc.gpsimd.memset(triu_f, 0.0)
            nc.gpsimd.affine_select(
                out=triu_f, in_=triu_f, compare_op=mybir.AluOpType.is_gt,
                fill=1.0, base=0, pattern=[[-1, P]], channel_multiplier=1)
            tri_bf = const.tile([P, P], BF16)
            nc.vector.tensor_copy(tri_bf, triu_f)
            triu_r = const.tile([P, P], F32R)
            nc.vector.tensor_copy(triu_r, triu_f)
            onesf = const.tile([P, P], F32)
            nc.vector.memset(onesf, 1.0)
            ones_r = const.tile([P, P], F32R)
            nc.vector.tensor_copy(ones_r, onesf)
            zeros8 = const.tile([P, NTB], F32)
            nc.vector.memset(zeros8, 0.0)
            # step masks: sm[j][p, m] = 1 iff m > j (offset matmul lhsT)
            sm_f = const.tile([P, NTB - 1, NTB], F32)
            sm_bf = const.tile([P, NTB - 1, NTB], BF16)
            nc.gpsimd.memset(sm_f, 1.0)
            for j in range(NTB - 1):
                nc.gpsimd.affine_select(
                    out=sm_f[:, j, :], in_=sm_f[:, j, :],
                    compare_op=mybir.AluOpType.is_gt,
                    fill=0.0, base=-j, pattern=[[1, NTB]], channel_multiplier=0)
            nc.vector.tensor_copy(sm_bf, sm_f)
            # row selectors: sel8[p, i*P+m] = 1 iff p == i (broadcast matmuls)
            sel_f = const.tile([NTB, NTB * P], F32)
            sel_bf = const.tile([NTB, NTB * P], BF16)
            nc.gpsimd.memset(sel_f, 1.0)
            nc.gpsimd.affine_select(
                out=sel_f, in_=sel_f, compare_op=mybir.AluOpType.is_equal,
                fill=0.0, base=0, pattern=[[-1, NTB], [0, P]],
                channel_multiplier=1)
            nc.vector.tensor_copy(sel_bf, sel_f)

            # ramp the PE clock gate while the first DMAs are in flight
            ps_warm = ps_P_pool.tile([P, P], F32, tag="pf", name="ps_warm")
            NWARM = 14
            for n in range(NWARM):
                nc.tensor.matmul(ps_warm, ones_r, ones_r,
                                 start=(n == 0), stop=(n == NWARM - 1))

            loop_ctx = (tc.For_i(0, loop_n, 1) if loop_n
                        else contextlib.nullcontext())
            with loop_ctx:
                # ---- all input DMAs up front: xT (needed first) on both
                # HWDGE queues, then x (needed in phase C) ----
                xts, xpairs = [], []
                for q in range(B_LOC * NPAIR):
                    i0 = 2 * q
                    xT2 = xtp.tile([P, 2, F], BF16, name="xT2")
                    if q == 0:
                        nc.sync.dma_start(out=xT2[:, 0, :], in_=xt_d.ap()[0])
                        nc.sync.dma_start(out=xT2[:, 1, :], in_=xt_d.ap()[1])
                    else:
                        nc.sync.dma_start(
                            out=xT2,
                            in_=xt_d.ap()[i0:i0 + 2].rearrange("j p f -> p j f"))
                    xts.append(xT2)
                for q in range(B_LOC * NPAIR):
                    i0 = 2 * q
                    x2 = xp.tile([P, 2, F], BF16, name="x2")
                    nc.gpsimd.dma_start(
                        out=x2,
                        in_=x_d.ap()[i0:i0 + 2].rearrange("j p f -> p j f"))
                    xpairs.append(x2)

                w_all_b, rec_b, lws_b, trws_b = [], [], [], []
                cumB_b = [None, None]
                c2_b = [None, None]
                m_all_b = []
                if have_mask:
                    for b in range(B_LOC):
                        m_all = wp.tile([P, NTB], F32, tag="m_all",
                                        name="m_all")
                        nc.sync.dma_start(out=m_all, in_=m_d.ap()[b])
                        m_all_b.append(m_all)

                def emit_phase_a(b):
                    """logits for batch b: per pair matmul->tanh->mul->fold->
                    reduce->exp, with tri/step scales issued per pair."""
                    alpha = wp.tile([P, NTB], BF16, tag="alpha", name="alpha")
                    w_all = wp.tile([P, NTB], F32, tag="w_all", name="w_all")
                    w_all_b.append(w_all)
                    lws, trws = [], []
                    lws_b.append(lws)
                    trws_b.append(trws)
                    for pp in range(NPAIR):
                        ps_v2 = ps_v_pool.tile([P, 2, F], F32, name="ps_v2")
                        for j in range(2):
                            t = NTB * b + 2 * pp + j
                            xT2 = xts[t // 2]
                            for k in range(KC):
                                nc.tensor.matmul(
                                    ps_v2[:, j, :],
                                    xT2[:, t % 2, k * P:(k + 1) * P],
                                    W_sb[:, k, :],
                                    start=(k == 0),
                                    stop=(k == KC - 1 and not have_b),
                                )
                            if have_b:
                                nc.tensor.matmul(ps_v2[:, j, :], ones_row,
                                                 b_sb, start=False, stop=True)
                        v2 = vp.tile([P, 2, F], BF16, name="v2")
                        nc.scalar.activation(out=v2, in_=ps_v2, func=Tanh)
                        scr2 = scrp.tile([P, 2, F], BF16, name="scr2")
                        nc.vector.tensor_mul(scr2, v2, u_bc2)
                        fld = foldp.tile([P, 2, F // 2], BF16, name="fld")
                        nc.vector.tensor_tensor(
                            out=fld, in0=scr2[:, :, 0:F // 2],
                            in1=scr2[:, :, F // 2:F], op=ADD)
                        sl = slice(2 * pp, 2 * pp + 2)
                        with nc.allow_low_precision(
                                reason="bf16 logits; 2e-2 rel-err budget"):
                            nc.vector.tensor_reduce(
                                alpha[:, sl], fld,
                                axis=mybir.AxisListType.X, op=ADD)
                        nc.scalar.activation(out=w_all[:, sl],
                                             in_=alpha[:, sl], func=Exp)
                        if have_mask:
                            nc.vector.tensor_mul(w_all[:, sl], w_all[:, sl],
                                                 m_all_b[b][:, sl])
                        for i in (2 * pp, 2 * pp + 1):
                            if i < NTB - 1:
                                lw = lwp.tile([P, NTB], BF16, name="lw")
                                nc.gpsimd.tensor_scalar_mul(
                                    lw, sm_bf[:, i, :], w_all[:, i:i + 1])
                                lws.append(lw)
                            trw = trwp.tile([P, P], BF16, name="trw")
                            nc.gpsimd.tensor_scalar_mul(
                                trw, tri_bf, w_all[:, i:i + 1])
                            trws.append(trw)

                def emit_zrec(b):
                    w_r = wp.tile([P, NTB], F32R, tag="w_r", name="w_r")
                    nc.vector.tensor_copy(w_r, w_all_b[b])
                    ps_z = ps_P_pool.tile([P, 2 * NTB], F32, tag="pf",
                                          name="ps_z")
                    nc.tensor.matmul(ps_z[:, 0:NTB], triu_r, w_r,
                                     start=True, stop=True)
                    nc.tensor.matmul(ps_z[:, NTB:2 * NTB], ones_r, w_r,
                                     start=True, stop=True)
                    S = wp.tile([P, NTB], F32, tag="S", name="S")
                    nc.vector.tensor_tensor_scan(
                        out=S, data0=ps_z[:, NTB:2 * NTB], data1=zeros8,
                        initial=0.0, op0=ADD, op1=ADD)
                    Z = wp.tile([P, NTB], F32, tag="Z", name="Z")
                    nc.vector.tensor_tensor(out=Z, in0=S,
                                            in1=ps_z[:, NTB:2 * NTB], op=SUB)
                    nc.vector.tensor_tensor(out=Z, in0=Z, in1=ps_z[:, 0:NTB],
                                            op=ADD)
                    nc.vector.tensor_scalar_add(Z, Z, EPS)
                    rec = wp.tile([P, NTB], F32, tag="rec", name="rec")
                    nc.vector.reciprocal(rec, Z)
                    rec_b.append(rec)

                def emit_offsets(b):
                    # ps_cum[m, f] = sum_{j<m} (w * x) tile totals for batch b
                    ps_cum = ps_P_pool.tile([NTB, F], F32, tag="pf",
                                            name="ps_cum")
                    for j in range(NTB - 1):
                        nc.tensor.matmul(ps_cum, lws_b[b][j],
                                         xpairs[NPAIR * b + j // 2][:, j % 2, :],
                                         start=(j == 0), stop=(j == NTB - 2))
                    cumB = cumbp.tile([NTB, F], BF16, name="cumB")
                    nc.scalar.activation(out=cumB, in_=ps_cum, func=Copy)
                    cumB_b[b] = cumB

                ro_pat = [0, 1] * 8
                ro_n = [0]

                def emit_ctile(b, i):
                    ps_P = ps_P_pool.tile([P, F], F32, tag="pf", name="ps_P")
                    nc.tensor.matmul(ps_P, trws_b[b][i],
                                     xpairs[NPAIR * b + i // 2][:, i % 2, :],
                                     start=True, stop=(i == 0))
                    if i > 0:
                        nc.tensor.matmul(ps_P, sel_bf[:, i * P:(i + 1) * P],
                                         cumB_b[b], start=False, stop=True)
                    rec = rec_b[b]
                    if i % 2 == 0:
                        c2_b[b] = cp.tile([P, 2, F], BF16, name="c2")
                    c_sl = c2_b[b][:, i % 2, :]
                    if ro_pat[ro_n[0]] == 0:
                        nc.scalar.activation(out=c_sl, in_=ps_P, func=Copy,
                                             scale=rec[:, i:i + 1])
                    else:
                        nc.vector.tensor_scalar_mul(c_sl, ps_P,
                                                    rec[:, i:i + 1])
                    ro_n[0] += 1
                    i0 = NTB * b + i
                    if b == 1 and i >= NTB - 2:
                        nc.sync.dma_start(out=c_d.ap()[i0], in_=c_sl)
                    elif i % 2 == 1:
                        nc.sync.dma_start(
                            out=c_d.ap()[i0 - 1:i0 + 1].rearrange(
                                "j p f -> p j f"),
                            in_=c2_b[b])

                emit_phase_a(0)
                emit_phase_a(1)
                emit_zrec(0)
                emit_offsets(0)
                for i in range(6):
                    emit_ctile(0, i)
                emit_zrec(1)
                emit_offsets(1)
                seq = [(0, 6), (1, 0), (0, 7), (1, 1), (1, 2), (1, 3),
                       (1, 4), (1, 5), (1, 6), (1, 7)]
                for b, i in seq:
                    emit_ctile(b, i)

    nc.compile()
    return nc


_NC_CACHE: dict = {}


def _get_nc(have_b, have_mask, loop_n=0):
    key = (have_b, have_mask, loop_n)
    if key not in _NC_CACHE:
        _NC_CACHE[key] = _build(have_b, have_mask, loop_n)
    return _NC_CACHE[key]


def _host_xt(xs):
    """xs: (NT, P, F) tile-major core shard -> pre-transposed layout where
    xt[i, p, k*128+t] = xs[i, t, k*128+p] (chunk-transposed for matmul lhsT)."""
    v = xs.reshape(NT, P, KC, P).transpose(0, 3, 2, 1)
    return np.ascontiguousarray(v).reshape(NT, P, F)


def make_core_maps(x, W, u, b=None, mask_f=None):
    """Build the 8 per-core input maps from full inputs."""
    x16 = x.astype(NPBF16)
    # W_host[p, k*F + f] = W[k*P + p, f]
    W_r = np.ascontiguousarray(
        W.reshape(KC, P, F).transpose(1, 0, 2).reshape(P, KC * F)).astype(NPBF16)
    u_r = np.ascontiguousarray(u.reshape(1, F)).astype(NPBF16)
    maps = []
    for core in range(NCORES):
        xs = np.ascontiguousarray(
            x16[core * B_LOC:(core + 1) * B_LOC].reshape(NT, P, F))
        m = {"x": xs, "xT": _host_xt(xs), "W": W_r, "u": u_r}
        if b is not None:
            m["b"] = np.ascontiguousarray(b.reshape(1, F)).astype(NPBF16)
        if mask_f is not None:
            ms = mask_f[core * B_LOC:(core + 1) * B_LOC]
            m["m"] = np.ascontiguousarray(
                ms.reshape(B_LOC, NTB, P).transpose(0, 2, 1))
        maps.append(m)
    return maps


def kernel(x, mask, W, b, u):
    x = np.asarray(x, dtype=np.float32)
    W = np.asarray(W, dtype=np.float32)
    b = np.asarray(b, dtype=np.float32)
    u = np.asarray(u, dtype=np.float32)
    mask_f = np.asarray(mask).astype(np.float32)

    have_b = bool(np.any(b != 0.0))
    have_mask = bool(np.any(mask_f != 1.0))

    nc = _get_nc(have_b, have_mask)
    in_maps = make_core_maps(x, W, u,
                             b if have_b else None,
                             mask_f if have_mask else None)
    res = run_bass_kernel_spmd(nc, in_maps, core_ids=list(range(NCORES)))
    out = np.stack([np.asarray(r["c"]).astype(np.float32).reshape(B_LOC, T, F)
                    for r in res.results])
    return out.reshape(B, T, F)


# revision 48
# speedup vs baseline: 2.3555x; 1.1374x over previous
"""Trainium2 Bass kernel for nn_Attention (cumulative masked softmax attention).

Reference computation:
    v   = tanh(x @ W + b)                  (B, T, F)
    a   = v . u                            (B, T)   -- query-independent logits
    e   = exp(a)[:, None, :] * tril * mask (B, T, T)
    alf = e / (sum_s e + EPS)
    c   = alf @ x                          (B, T, F)

Because the logits are query-independent and the mask is lower-triangular,
the (B,T,T) softmax-matmul collapses to a running weighted average:
    w[s]  = exp(a[s]) * mask[s]
    c[t]  = cumsum_s(w * x)[t] / (cumsum_s(w)[t] + EPS)
which is O(B*T*F) instead of O(B*T^2*F).

Sharding: data-parallel over batch B across 8 NeuronCores (2 batches/core).

v3 design (bf16):
  - All HBM traffic in bf16 (x, xT, W, u, c out) -- rel-err budget is 2e-2,
    bf16 keeps it ~2e-3 while halving DMA bytes.
  - The weights w fold into the 128x128 triangular matrix (tri_w = tri * w
    per tile, a cheap [128,128] gpsimd scale) instead of scaling x.
  - Cross-tile prefix offsets via 7 "step-mask" matmuls per batch (lhsT
    column m gets w[:,j] iff m > j) writing the per-tile offset rows [8, F]
    in PSUM; one copy to SBUF, then one [8,128]-selector matmul per tile
    broadcasts its offset row onto the tile prefix.
  - Denominator Z = prefix(w) via two tiny [P,8] matmuls + a free-dim scan
    (f32), reciprocal folded into the PSUM->SBUF readout scale.
  - Scheduling: xT loads split across the SP and ACT DMA queues ahead of
    the x loads; exp/tri-scales issued per pair so phase C of batch 0 can
    fill the tensor engine while batch 1's logits chain completes.
"""

import contextlib

import numpy as np
import ml_dtypes

import concourse.bass as bass  # noqa: F401
import concourse.tile as tile
from concourse import bacc, mybir
from concourse.bass_utils import run_bass_kernel_spmd

B, T, F = 16, 1024, 512
EPS = 1e-7
NCORES = 8
B_LOC = B // NCORES          # batches per core
R = B_LOC * T                # rows per core
P = 128                      # partition tile
NT = R // P                  # row tiles per core
NTB = T // P                 # row tiles per batch
NPAIR = NTB // 2             # tile pairs per batch
KC = F // P                  # contraction chunks

F32 = mybir.dt.float32
F32R = mybir.dt.float32r
BF16 = mybir.dt.bfloat16
NPBF16 = ml_dtypes.bfloat16


def _build(have_b: bool, have_mask: bool, loop_n: int = 0):
    """Build the per-core Bass module. loop_n > 0 wraps the body in a
    hardware For_i loop (used only for timing)."""
    nc = bacc.Bacc("TRN2", target_bir_lowering=False, debug=False)

    x_d = nc.dram_tensor("x", [NT, P, F], BF16, kind="ExternalInput")
    xt_d = nc.dram_tensor("xT", [NT, P, F], BF16, kind="ExternalInput")
    # W pre-arranged on host as (P, KC*F): W_host[p, k*F+f] = W[k*P+p, f]
    w_d = nc.dram_tensor("W", [P, KC * F], BF16, kind="ExternalInput")
    u_d = nc.dram_tensor("u", [1, F], BF16, kind="ExternalInput")
    if have_b:
        b_d = nc.dram_tensor("b", [1, F], BF16, kind="ExternalInput")
    if have_mask:
        m_d = nc.dram_tensor("m", [B_LOC, P, NTB], F32, kind="ExternalInput")
    c_d = nc.dram_tensor("c", [NT, P, F], BF16, kind="ExternalOutput")

    Tanh = mybir.ActivationFunctionType.Tanh
    Exp = mybir.ActivationFunctionType.Exp
    Copy = mybir.ActivationFunctionType.Copy
    ADD = mybir.AluOpType.add
    SUB = mybir.AluOpType.subtract

    with tile.TileContext(nc) as tc:
        with (
            tc.tile_pool(name="const", bufs=1) as const,
            tc.tile_pool(name="xp", bufs=2 * NPAIR) as xp,
            tc.tile_pool(name="xtp", bufs=2 * NPAIR) as xtp,
            tc.tile_pool(name="vp", bufs=2) as vp,
            tc.tile_pool(name="scrp", bufs=2) as scrp,
            tc.tile_pool(name="foldp", bufs=2) as foldp,
            tc.tile_pool(name="wp", bufs=2) as wp,
            tc.tile_pool(name="trwp", bufs=10) as trwp,
            tc.tile_pool(name="lwp", bufs=8) as lwp,
            tc.tile_pool(name="cumbp", bufs=2) as cumbp,
            tc.tile_pool(name="cp", bufs=6) as cp,
            tc.tile_pool(name="ps_v", bufs=2, space="PSUM") as ps_v_pool,
            tc.tile_pool(name="ps_P", bufs=4, space="PSUM") as ps_P_pool,
        ):
            # ---- constants ----
            # W split into per-chunk DMAs on the gpsimd queue so the first
            # matmul only waits on chunk 0 (+ the first xT tile) and the SP
            # queue can issue the xT loads immediately.
            W_sb = const.tile([P, KC, F], BF16)
            wr_ap = w_d.ap().rearrange("p (k f) -> p k f", k=KC)
            nc.scalar.dma_start(out=W_sb, in_=wr_ap)
            u_bc2 = const.tile([P, 2, F], BF16)
            nc.gpsimd.dma_start(out=u_bc2[:, 0, :],
                                in_=u_d.ap().to_broadcast((P, F)))
            nc.gpsimd.dma_start(out=u_bc2[:, 1, :],
                                in_=u_d.ap().to_broadcast((P, F)))
            onesf = const.tile([P, P], F32)
            nc.vector.memset(onesf, 1.0)
            ones_r = const.tile([P, P], F32R)
            nc.vector.tensor_copy(ones_r, onesf)

            if have_b:
                b_sb = const.tile([1, F], BF16)
                nc.sync.dma_start(out=b_sb, in_=b_d.ap())
                ones_row = const.tile([1, P], BF16)
                nc.vector.memset(ones_row, 1.0)

            # triangular matrices: triu[p, m] = 1 iff p <= m (inclusive prefix)
            triu_f = const.tile([P, P], F32)
            nc.gpsimd.memset(triu_f, 0.0)
            nc.gpsimd.affine_select(
                out=triu_f, in_=triu_f, compare_op=mybir.AluOpType.is_gt,
                fill=1.0, base=0, pattern=[[-1, P]], channel_multiplier=1)
            tri_bf = const.tile([P, P], BF16)
            nc.vector.tensor_copy(tri_bf, triu_f)
            triu_r = const.tile([P, P], F32R)
            nc.vector.tensor_copy(triu_r, triu_f)
            zeros8 = const.tile([P, NTB], F32)
            nc.vector.memset(zeros8, 0.0)
            # step masks: sm[j][p, m] = 1 iff m > j (offset matmul lhsT)
            sm_f = const.tile([P, NTB - 1, NTB], F32)
            sm_bf = const.tile([P, NTB - 1, NTB], BF16)
            nc.gpsimd.memset(sm_f, 1.0)
            for j in range(NTB - 1):
                nc.gpsimd.affine_select(
                    out=sm_f[:, j, :], in_=sm_f[:, j, :],
                    compare_op=mybir.AluOpType.is_gt,
                    fill=0.0, base=-j, pattern=[[1, NTB]], channel_multiplier=0)
            nc.vector.tensor_copy(sm_bf, sm_f)
            # row selectors: sel8[p, i*P+m] = 1 iff p == i (broadcast matmuls)
            sel_f = const.tile([NTB, NTB * P], F32)
            sel_bf = const.tile([NTB, NTB * P], BF16)
            nc.gpsimd.memset(sel_f, 1.0)
            nc.gpsimd.affine_select(
                out=sel_f, in_=sel_f, compare_op=mybir.AluOpType.is_equal,
                fill=0.0, base=0, pattern=[[-1, NTB], [0, P]],
                channel_multiplier=1)
            nc.vector.tensor_copy(sel_bf, sel_f)

            # ramp the PE clock gate while the first DMAs are in flight
            ps_warm = ps_P_pool.tile([P, P], F32, tag="pf", name="ps_warm")
            NWARM = 10
            for n in range(NWARM):
                nc.tensor.matmul(ps_warm, ones_r, ones_r,
                                 start=(n == 0), stop=(n == NWARM - 1))

            loop_ctx = (tc.For_i(0, loop_n, 1) if loop_n
                        else contextlib.nullcontext())
            with loop_ctx:
                # ---- all input DMAs up front: xT (needed first) on both
                # HWDGE queues, then x (needed in phase C) ----
                xts, xpairs = [], []
                for q in range(B_LOC * NPAIR):
                    i0 = 2 * q
                    # the last two pairs ride the ACT queue: their transfers
                    # finish before ACT's first tanh input is even ready, so
                    # they never block ACT compute but double the load cadence
                    eng = nc.scalar if q in (1, 6) else nc.sync
                    xT2 = xtp.tile([P, 2, F], BF16, name="xT2")
                    if q == 0:
                        nc.sync.dma_start(out=xT2[:, 0, :], in_=xt_d.ap()[0])
                        nc.sync.dma_start(out=xT2[:, 1, :], in_=xt_d.ap()[1])
                    else:
                        eng.dma_start(
                            out=xT2,
                            in_=xt_d.ap()[i0:i0 + 2].rearrange("j p f -> p j f"))
                    xts.append(xT2)
                for q in range(B_LOC * NPAIR):
                    i0 = 2 * q
                    x2 = xp.tile([P, 2, F], BF16, name="x2")
                    nc.gpsimd.dma_start(
                        out=x2,
                        in_=x_d.ap()[i0:i0 + 2].rearrange("j p f -> p j f"))
                    xpairs.append(x2)

                w_all_b, rec_b, lws_b, trws_b = [], [], [], []
                cumB_b = [None, None]
                c2_b = [None, None]
                m_all_b = []
                if have_mask:
                    for b in range(B_LOC):
                        m_all = wp.tile([P, NTB], F32, tag="m_all",
                                        name="m_all")
                        nc.sync.dma_start(out=m_all, in_=m_d.ap()[b])
                        m_all_b.append(m_all)

                def emit_phase_a(b):
                    """logits for batch b: per pair matmul->tanh->mul->fold->
                    reduce->exp, with tri/step scales issued per pair."""
                    alpha = wp.tile([P, NTB], BF16, tag="alpha", name="alpha")
                    w_all = wp.tile([P, NTB], F32, tag="w_all", name="w_all")
                    w_all_b.append(w_all)
                    lws, trws = [], []
                    lws_b.append(lws)
                    trws_b.append(trws)
                    for pp in range(NPAIR):
                        ps_v2 = ps_v_pool.tile([P, 2, F], F32, name="ps_v2")
                        for j in range(2):
                            t = NTB * b + 2 * pp + j
                            xT2 = xts[t // 2]
                            for k in range(KC):
                                nc.tensor.matmul(
                                    ps_v2[:, j, :],
                                    xT2[:, t % 2, k * P:(k + 1) * P],
                                    W_sb[:, k, :],
                                    start=(k == 0),
                                    stop=(k == KC - 1 and not have_b),
                                )
                            if have_b:
                                nc.tensor.matmul(ps_v2[:, j, :], ones_row,
                                                 b_sb, start=False, stop=True)
                        v2 = vp.tile([P, 2, F], BF16, name="v2")
                        nc.scalar.activation(out=v2, in_=ps_v2, func=Tanh)
                        scr2 = scrp.tile([P, 2, F], BF16, name="scr2")
                        nc.vector.tensor_mul(scr2, v2, u_bc2)
                        fld = foldp.tile([P, 2, F // 2], BF16, name="fld")
                        nc.vector.tensor_tensor(
                            out=fld, in0=scr2[:, :, 0:F // 2],
                            in1=scr2[:, :, F // 2:F], op=ADD)
                        sl = slice(2 * pp, 2 * pp + 2)
                        with nc.allow_low_precision(
                                reason="bf16 logits; 2e-2 rel-err budget"):
                            nc.vector.tensor_reduce(
                                alpha[:, sl], fld,
                                axis=mybir.AxisListType.X, op=ADD)
                        nc.scalar.activation(out=w_all[:, sl],
                                             in_=alpha[:, sl], func=Exp)
                        if have_mask:
                            nc.vector.tensor_mul(w_all[:, sl], w_all[:, sl],
                                                 m_all_b[b][:, sl])
                        for i in (2 * pp, 2 * pp + 1):
                            if i < NTB - 1:
                                lw = lwp.tile([P, NTB], BF16, name="lw")
                                nc.gpsimd.tensor_scalar_mul(
                                    lw, sm_bf[:, i, :], w_all[:, i:i + 1])
                                lws.append(lw)
                            trw = trwp.tile([P, P], BF16, name="trw")
                            nc.gpsimd.tensor_scalar_mul(
                                trw, tri_bf, w_all[:, i:i + 1])
                            trws.append(trw)

                def emit_zrec(b):
                    w_r = wp.tile([P, NTB], F32R, tag="w_r", name="w_r")
                    nc.vector.tensor_copy(w_r, w_all_b[b])
                    ps_z = ps_P_pool.tile([P, 2 * NTB], F32, tag="pf",
                                          name="ps_z")
                    nc.tensor.matmul(ps_z[:, 0:NTB], triu_r, w_r,
                                     start=True, stop=True)
                    nc.tensor.matmul(ps_z[:, NTB:2 * NTB], ones_r, w_r,
                                     start=True, stop=True)
                    S = wp.tile([P, NTB], F32, tag="S", name="S")
                    nc.vector.tensor_tensor_scan(
                        out=S, data0=ps_z[:, NTB:2 * NTB], data1=zeros8,
                        initial=0.0, op0=ADD, op1=ADD)
                    Z = wp.tile([P, NTB], F32, tag="Z", name="Z")
                    nc.vector.tensor_tensor(out=Z, in0=S,
                                            in1=ps_z[:, NTB:2 * NTB], op=SUB)
                    nc.vector.tensor_tensor(out=Z, in0=Z, in1=ps_z[:, 0:NTB],
                                            op=ADD)
                    nc.vector.tensor_scalar_add(Z, Z, EPS)
                    rec = wp.tile([P, NTB], F32, tag="rec", name="rec")
                    nc.vector.reciprocal(rec, Z)
                    rec_b.append(rec)

                def emit_offsets(b):
                    # ps_cum[m, f] = sum_{j<m} (w * x) tile totals for batch b
                    ps_cum = ps_P_pool.tile([NTB, F], F32, tag="pf",
                                            name="ps_cum")
                    for j in range(NTB - 1):
                        nc.tensor.matmul(ps_cum, lws_b[b][j],
                                         xpairs[NPAIR * b + j // 2][:, j % 2, :],
                                         start=(j == 0), stop=(j == NTB - 2))
                    cumB = cumbp.tile([NTB, F], BF16, name="cumB")
                    nc.scalar.activation(out=cumB, in_=ps_cum, func=Copy)
                    cumB_b[b] = cumB

                ro_pat = [0, 1] * 8
                ro_n = [0]

                def emit_ctile(b, i):
                    ps_P = ps_P_pool.tile([P, F], F32, tag="pf", name="ps_P")
                    nc.tensor.matmul(ps_P, trws_b[b][i],
                                     xpairs[NPAIR * b + i // 2][:, i % 2, :],
                                     start=True, stop=(i == 0))
                    if i > 0:
                        nc.tensor.matmul(ps_P, sel_bf[:, i * P:(i + 1) * P],
                                         cumB_b[b], start=False, stop=True)
                    rec = rec_b[b]
                    if i % 2 == 0:
                        c2_b[b] = cp.tile([P, 2, F], BF16, name="c2")
                    c_sl = c2_b[b][:, i % 2, :]
                    if ro_pat[ro_n[0]] == 0:
                        nc.scalar.activation(out=c_sl, in_=ps_P, func=Copy,
                                             scale=rec[:, i:i + 1])
                    else:
                        nc.vector.tensor_scalar_mul(c_sl, ps_P,
                                                    rec[:, i:i + 1])
                    ro_n[0] += 1
                    i0 = NTB * b + i
                    if b == 1 and i >= NTB - 2:
                        nc.sync.dma_start(out=c_d.ap()[i0], in_=c_sl)
                    elif i % 2 == 1:
                        nc.sync.dma_start(
                            out=c_d.ap()[i0 - 1:i0 + 1].rearrange(
                                "j p f -> p j f"),
                            in_=c2_b[b])

                emit_phase_a(0)
                emit_phase_a(1)
                emit_zrec(0)
                emit_offsets(0)
                for i in range(6):
                    emit_ctile(0, i)
                emit_zrec(1)
                emit_offsets(1)
                seq = [(0, 6), (1, 0), (0, 7), (1, 1), (1, 2), (1, 3),
                       (1, 4), (1, 5), (1, 6), (1, 7)]
                for b, i in seq:
                    emit_ctile(b, i)

    nc.compile()
    return nc


_NC_CACHE: dict = {}


def _get_nc(have_b, have_mask, loop_n=0):
    key = (have_b, have_mask, loop_n)
    if key not in _NC_CACHE:
        _NC_CACHE[key] = _build(have_b, have_mask, loop_n)
    return _NC_CACHE[key]


def _host_xt(xs):
    """xs: (NT, P, F) tile-major core shard -> pre-transposed layout where
    xt[i, p, k*128+t] = xs[i, t, k*128+p] (chunk-transposed for matmul lhsT)."""
    v = xs.reshape(NT, P, KC, P).transpose(0, 3, 2, 1)
    return np.ascontiguousarray(v).reshape(NT, P, F)


def make_core_maps(x, W, u, b=None, mask_f=None):
    """Build the 8 per-core input maps from full inputs."""
    x16 = x.astype(NPBF16)
    # W_host[p, k*F + f] = W[k*P + p, f]
    W_r = np.ascontiguousarray(
        W.reshape(KC, P, F).transpose(1, 0, 2).reshape(P, KC * F)).astype(NPBF16)
    u_r = np.ascontiguousarray(u.reshape(1, F)).astype(NPBF16)
    maps = []
    for core in range(NCORES):
        xs = np.ascontiguousarray(
            x16[core * B_LOC:(core + 1) * B_LOC].reshape(NT, P, F))
        m = {"x": xs, "xT": _host_xt(xs), "W": W_r, "u": u_r}
        if b is not None:
            m["b"] = np.ascontiguousarray(b.reshape(1, F)).astype(NPBF16)
        if mask_f is not None:
            ms = mask_f[core * B_LOC:(core + 1) * B_LOC]
            m["m"] = np.ascontiguousarray(
                ms.reshape(B_LOC, NTB, P).transpose(0, 2, 1))
        maps.append(m)
    return maps


def kernel(x, mask, W, b, u):
    x = np.asarray(x, dtype=np.float32)
    W = np.asarray(W, dtype=np.float32)
    b = np.asarray(b, dtype=np.float32)
    u = np.asarray(u, dtype=np.float32)
    mask_f = np.asarray(mask).astype(np.float32)

    have_b = bool(np.any(b != 0.0))
    have_mask = bool(np.any(mask_f != 1.0))

    nc = _get_nc(have_b, have_mask)
    in_maps = make_core_maps(x, W, u,
                             b if have_b else None,
                             mask_f if have_mask else None)
    res = run_bass_kernel_spmd(nc, in_maps, core_ids=list(range(NCORES)))
    out = np.stack([np.asarray(r["c"]).astype(np.float32).reshape(B_LOC, T, F)
                    for r in res.results])
    return out.reshape(B, T, F)


# revision 51
# speedup vs baseline: 2.3750x; 1.0083x over previous
"""Trainium2 Bass kernel for nn_Attention (cumulative masked softmax attention).

Reference computation:
    v   = tanh(x @ W + b)                  (B, T, F)
    a   = v . u                            (B, T)   -- query-independent logits
    e   = exp(a)[:, None, :] * tril * mask (B, T, T)
    alf = e / (sum_s e + EPS)
    c   = alf @ x                          (B, T, F)

Because the logits are query-independent and the mask is lower-triangular,
the (B,T,T) softmax-matmul collapses to a running weighted average:
    w[s]  = exp(a[s]) * mask[s]
    c[t]  = cumsum_s(w * x)[t] / (cumsum_s(w)[t] + EPS)
which is O(B*T*F) instead of O(B*T^2*F).

Sharding: data-parallel over batch B across 8 NeuronCores (2 batches/core).

v3 design (bf16):
  - All HBM traffic in bf16 (x, xT, W, u, c out) -- rel-err budget is 2e-2,
    bf16 keeps it ~2e-3 while halving DMA bytes.
  - The weights w fold into the 128x128 triangular matrix (tri_w = tri * w
    per tile, a cheap [128,128] gpsimd scale) instead of scaling x.
  - Cross-tile prefix offsets via 7 "step-mask" matmuls per batch (lhsT
    column m gets w[:,j] iff m > j) writing the per-tile offset rows [8, F]
    in PSUM; one copy to SBUF, then one [8,128]-selector matmul per tile
    broadcasts its offset row onto the tile prefix.
  - Denominator Z = prefix(w) via two tiny [P,8] matmuls + a free-dim scan
    (f32), reciprocal folded into the PSUM->SBUF readout scale.
  - Scheduling: xT loads split across the SP and ACT DMA queues ahead of
    the x loads; exp/tri-scales issued per pair so phase C of batch 0 can
    fill the tensor engine while batch 1's logits chain completes.
"""

import contextlib

import numpy as np
import ml_dtypes

import concourse.bass as bass  # noqa: F401
import concourse.tile as tile
from concourse import bacc, mybir
from concourse.bass_utils import run_bass_kernel_spmd

B, T, F = 16, 1024, 512
EPS = 1e-7
NCORES = 8
B_LOC = B // NCORES          # batches per core
R = B_LOC * T                # rows per core
P = 128                      # partition tile
NT = R // P                  # row tiles per core
NTB = T // P                 # row tiles per batch
NPAIR = NTB // 2             # tile pairs per batch
KC = F // P                  # contraction chunks

F32 = mybir.dt.float32
F32R = mybir.dt.float32r
BF16 = mybir.dt.bfloat16
NPBF16 = ml_dtypes.bfloat16


def _build(have_b: bool, have_mask: bool, loop_n: int = 0):
    """Build the per-core Bass module. loop_n > 0 wraps the body in a
    hardware For_i loop (used only for timing)."""
    nc = bacc.Bacc("TRN2", target_bir_lowering=False, debug=False)

    x_d = nc.dram_tensor("x", [NT, P, F], BF16, kind="ExternalInput")
    xt_d = nc.dram_tensor("xT", [NT, P, F], BF16, kind="ExternalInput")
    # W pre-arranged on host as (P, KC*F): W_host[p, k*F+f] = W[k*P+p, f]
    w_d = nc.dram_tensor("W", [P, KC * F], BF16, kind="ExternalInput")
    u_d = nc.dram_tensor("u", [1, F], BF16, kind="ExternalInput")
    if have_b:
        b_d = nc.dram_tensor("b", [1, F], BF16, kind="ExternalInput")
    if have_mask:
        m_d = nc.dram_tensor("m", [B_LOC, P, NTB], F32, kind="ExternalInput")
    c_d = nc.dram_tensor("c", [NT, P, F], BF16, kind="ExternalOutput")

    Tanh = mybir.ActivationFunctionType.Tanh
    Exp = mybir.ActivationFunctionType.Exp
    Copy = mybir.ActivationFunctionType.Copy
    ADD = mybir.AluOpType.add
    SUB = mybir.AluOpType.subtract

    with tile.TileContext(nc) as tc:
        with (
            tc.tile_pool(name="const", bufs=1) as const,
            tc.tile_pool(name="xp", bufs=2 * NPAIR) as xp,
            tc.tile_pool(name="xtp", bufs=2 * NPAIR) as xtp,
            tc.tile_pool(name="vp", bufs=2) as vp,
            tc.tile_pool(name="scrp", bufs=2) as scrp,
            tc.tile_pool(name="foldp", bufs=2) as foldp,
            tc.tile_pool(name="wp", bufs=2) as wp,
            tc.tile_pool(name="trwp", bufs=10) as trwp,
            tc.tile_pool(name="lwp", bufs=8) as lwp,
            tc.tile_pool(name="cumbp", bufs=2) as cumbp,
            tc.tile_pool(name="cp", bufs=8) as cp,
            tc.tile_pool(name="ps_v", bufs=2, space="PSUM") as ps_v_pool,
            tc.tile_pool(name="ps_P", bufs=4, space="PSUM") as ps_P_pool,
        ):
            # ---- constants ----
            # W split into per-chunk DMAs on the gpsimd queue so the first
            # matmul only waits on chunk 0 (+ the first xT tile) and the SP
            # queue can issue the xT loads immediately.
            W_sb = const.tile([P, KC, F], BF16)
            wr_ap = w_d.ap().rearrange("p (k f) -> p k f", k=KC)
            nc.scalar.dma_start(out=W_sb, in_=wr_ap)
            u_bc2 = const.tile([P, 2, F], BF16)
            nc.gpsimd.dma_start(out=u_bc2[:, 0, :],
                                in_=u_d.ap().to_broadcast((P, F)))
            nc.gpsimd.dma_start(out=u_bc2[:, 1, :],
                                in_=u_d.ap().to_broadcast((P, F)))
            onesf = const.tile([P, P], F32)
            nc.vector.memset(onesf, 1.0)
            ones_r = const.tile([P, P], F32R)
            nc.vector.tensor_copy(ones_r, onesf)

            if have_b:
                b_sb = const.tile([1, F], BF16)
                nc.sync.dma_start(out=b_sb, in_=b_d.ap())
                ones_row = const.tile([1, P], BF16)
                nc.vector.memset(ones_row, 1.0)

            # triangular matrices: triu[p, m] = 1 iff p <= m (inclusive prefix)
            triu_f = const.tile([P, P], F32)
            nc.gpsimd.memset(triu_f, 0.0)
            nc.gpsimd.affine_select(
                out=triu_f, in_=triu_f, compare_op=mybir.AluOpType.is_gt,
                fill=1.0, base=0, pattern=[[-1, P]], channel_multiplier=1)
            tri_bf = const.tile([P, P], BF16)
            nc.vector.tensor_copy(tri_bf, triu_f)
            triu_r = const.tile([P, P], F32R)
            nc.vector.tensor_copy(triu_r, triu_f)
            zeros8 = const.tile([P, NTB], F32)
            nc.vector.memset(zeros8, 0.0)
            # step masks: sm[j][p, m] = 1 iff m > j (offset matmul lhsT)
            sm_f = const.tile([P, NTB - 1, NTB], F32)
            sm_bf = const.tile([P, NTB - 1, NTB], BF16)
            nc.gpsimd.memset(sm_f, 1.0)
            for j in range(NTB - 1):
                nc.gpsimd.affine_select(
                    out=sm_f[:, j, :], in_=sm_f[:, j, :],
                    compare_op=mybir.AluOpType.is_gt,
                    fill=0.0, base=-j, pattern=[[1, NTB]], channel_multiplier=0)
            nc.vector.tensor_copy(sm_bf, sm_f)
            # row selectors: sel8[p, i*P+m] = 1 iff p == i (broadcast matmuls)
            sel_f = const.tile([NTB, NTB * P], F32)
            sel_bf = const.tile([NTB, NTB * P], BF16)
            nc.gpsimd.memset(sel_f, 1.0)
            nc.gpsimd.affine_select(
                out=sel_f, in_=sel_f, compare_op=mybir.AluOpType.is_equal,
                fill=0.0, base=0, pattern=[[-1, NTB], [0, P]],
                channel_multiplier=1)
            nc.vector.tensor_copy(sel_bf, sel_f)

            # ramp the PE clock gate while the first DMAs are in flight
            ps_warm = ps_P_pool.tile([P, P], F32, tag="pf", name="ps_warm")
            NWARM = 10
            for n in range(NWARM):
                nc.tensor.matmul(ps_warm, ones_r, ones_r,
                                 start=(n == 0), stop=(n == NWARM - 1))

            loop_ctx = (tc.For_i(0, loop_n, 1) if loop_n
                        else contextlib.nullcontext())
            with loop_ctx:
                # ---- all input DMAs up front: xT (needed first) on both
                # HWDGE queues, then x (needed in phase C) ----
                xts, xpairs = [], []
                for q in range(B_LOC * NPAIR):
                    i0 = 2 * q
                    # the last two pairs ride the ACT queue: their transfers
                    # finish before ACT's first tanh input is even ready, so
                    # they never block ACT compute but double the load cadence
                    eng = nc.scalar if q in (1, 6) else nc.sync
                    xT2 = xtp.tile([P, 2, F], BF16, name="xT2")
                    if q == 0:
                        nc.sync.dma_start(out=xT2[:, 0, :], in_=xt_d.ap()[0])
                        nc.sync.dma_start(out=xT2[:, 1, :], in_=xt_d.ap()[1])
                    else:
                        eng.dma_start(
                            out=xT2,
                            in_=xt_d.ap()[i0:i0 + 2].rearrange("j p f -> p j f"))
                    xts.append(xT2)
                for q in range(B_LOC * NPAIR):
                    i0 = 2 * q
                    x2 = xp.tile([P, 2, F], BF16, name="x2")
                    nc.gpsimd.dma_start(
                        out=x2,
                        in_=x_d.ap()[i0:i0 + 2].rearrange("j p f -> p j f"))
                    xpairs.append(x2)

                w_all_b, rec_b, lws_b, trws_b = [], [], [], []
                cumB_b = [None, None]
                c2_b = [None, None]
                m_all_b = []
                if have_mask:
                    for b in range(B_LOC):
                        m_all = wp.tile([P, NTB], F32, tag="m_all",
                                        name="m_all")
                        nc.sync.dma_start(out=m_all, in_=m_d.ap()[b])
                        m_all_b.append(m_all)

                def emit_phase_a(b):
                    """logits for batch b: per pair matmul->tanh->mul->fold->
                    reduce->exp, with tri/step scales issued per pair."""
                    alpha = wp.tile([P, NTB], BF16, tag="alpha", name="alpha")
                    w_all = wp.tile([P, NTB], F32, tag="w_all", name="w_all")
                    w_all_b.append(w_all)
                    lws, trws = [], []
                    lws_b.append(lws)
                    trws_b.append(trws)
                    for pp in range(NPAIR):
                        ps_v2 = ps_v_pool.tile([P, 2, F], F32, name="ps_v2")
                        for j in range(2):
                            t = NTB * b + 2 * pp + j
                            xT2 = xts[t // 2]
                            for k in range(KC):
                                nc.tensor.matmul(
                                    ps_v2[:, j, :],
                                    xT2[:, t % 2, k * P:(k + 1) * P],
                                    W_sb[:, k, :],
                                    start=(k == 0),
                                    stop=(k == KC - 1 and not have_b),
                                )
                            if have_b:
                                nc.tensor.matmul(ps_v2[:, j, :], ones_row,
                                                 b_sb, start=False, stop=True)
                        v2 = vp.tile([P, 2, F], BF16, name="v2")
                        nc.scalar.activation(out=v2, in_=ps_v2, func=Tanh)
                        scr2 = scrp.tile([P, 2, F], BF16, name="scr2")
                        nc.vector.tensor_mul(scr2, v2, u_bc2)
                        fld = foldp.tile([P, 2, F // 2], BF16, name="fld")
                        nc.vector.tensor_tensor(
                            out=fld, in0=scr2[:, :, 0:F // 2],
                            in1=scr2[:, :, F // 2:F], op=ADD)
                        sl = slice(2 * pp, 2 * pp + 2)
                        with nc.allow_low_precision(
                                reason="bf16 logits; 2e-2 rel-err budget"):
                            nc.vector.tensor_reduce(
                                alpha[:, sl], fld,
                                axis=mybir.AxisListType.X, op=ADD)
                        nc.scalar.activation(out=w_all[:, sl],
                                             in_=alpha[:, sl], func=Exp)
                        if have_mask:
                            nc.vector.tensor_mul(w_all[:, sl], w_all[:, sl],
                                                 m_all_b[b][:, sl])
                        for i in (2 * pp, 2 * pp + 1):
                            if i < NTB - 1:
                                lw = lwp.tile([P, NTB], BF16, name="lw")
                                nc.gpsimd.tensor_scalar_mul(
                                    lw, sm_bf[:, i, :], w_all[:, i:i + 1])
                                lws.append(lw)
                            trw = trwp.tile([P, P], BF16, name="trw")
                            nc.gpsimd.tensor_scalar_mul(
                                trw, tri_bf, w_all[:, i:i + 1])
                            trws.append(trw)

                def emit_zrec(b):
                    w_r = wp.tile([P, NTB], F32R, tag="w_r", name="w_r")
                    nc.vector.tensor_copy(w_r, w_all_b[b])
                    ps_z = ps_P_pool.tile([P, 2 * NTB], F32, tag="pf",
                                          name="ps_z")
                    nc.tensor.matmul(ps_z[:, 0:NTB], triu_r, w_r,
                                     start=True, stop=True)
                    nc.tensor.matmul(ps_z[:, NTB:2 * NTB], ones_r, w_r,
                                     start=True, stop=True)
                    S = wp.tile([P, NTB], F32, tag="S", name="S")
                    nc.vector.tensor_tensor_scan(
                        out=S, data0=ps_z[:, NTB:2 * NTB], data1=zeros8,
                        initial=0.0, op0=ADD, op1=ADD)
                    Z = wp.tile([P, NTB], F32, tag="Z", name="Z")
                    nc.vector.tensor_tensor(out=Z, in0=S,
                                            in1=ps_z[:, NTB:2 * NTB], op=SUB)
                    nc.vector.tensor_tensor(out=Z, in0=Z, in1=ps_z[:, 0:NTB],
                                            op=ADD)
                    nc.vector.tensor_scalar_add(Z, Z, EPS)
                    rec = wp.tile([P, NTB], F32, tag="rec", name="rec")
                    nc.vector.reciprocal(rec, Z)
                    rec_b.append(rec)

                def emit_offsets(b):
                    # ps_cum[m, f] = sum_{j<m} (w * x) tile totals for batch b
                    ps_cum = ps_P_pool.tile([NTB, F], F32, tag="pf",
                                            name="ps_cum")
                    for j in range(NTB - 1):
                        nc.tensor.matmul(ps_cum, lws_b[b][j],
                                         xpairs[NPAIR * b + j // 2][:, j % 2, :],
                                         start=(j == 0), stop=(j == NTB - 2))
                    cumB = cumbp.tile([NTB, F], BF16, name="cumB")
                    nc.scalar.activation(out=cumB, in_=ps_cum, func=Copy)
                    cumB_b[b] = cumB

                ro_pat = [0, 1] * 8
                ro_n = [0]

                def emit_ctile(b, i):
                    ps_P = ps_P_pool.tile([P, F], F32, tag="pf", name="ps_P")
                    nc.tensor.matmul(ps_P, trws_b[b][i],
                                     xpairs[NPAIR * b + i // 2][:, i % 2, :],
                                     start=True, stop=(i == 0))
                    if i > 0:
                        nc.tensor.matmul(ps_P, sel_bf[:, i * P:(i + 1) * P],
                                         cumB_b[b], start=False, stop=True)
                    rec = rec_b[b]
                    if i % 2 == 0:
                        c2_b[b] = cp.tile([P, 2, F], BF16, name="c2")
                    c_sl = c2_b[b][:, i % 2, :]
                    if ro_pat[ro_n[0]] == 0:
                        nc.scalar.activation(out=c_sl, in_=ps_P, func=Copy,
                                             scale=rec[:, i:i + 1])
                    else:
                        nc.vector.tensor_scalar_mul(c_sl, ps_P,
                                                    rec[:, i:i + 1])
                    ro_n[0] += 1
                    i0 = NTB * b + i
                    if b == 1 and i >= NTB - 2:
                        nc.sync.dma_start(out=c_d.ap()[i0], in_=c_sl)
                    elif i % 2 == 1:
                        nc.sync.dma_start(
                            out=c_d.ap()[i0 - 1:i0 + 1].rearrange(
                                "j p f -> p j f"),
                            in_=c2_b[b])

                emit_phase_a(0)
                emit_phase_a(1)
                emit_zrec(0)
                emit_offsets(0)
                for i in range(7):
                    emit_ctile(0, i)
                emit_zrec(1)
                emit_offsets(1)
                seq = [(0, 7), (1, 0), (1, 1), (1, 2), (1, 3),
                       (1, 4), (1, 5), (1, 6), (1, 7)]
                for b, i in seq:
                    emit_ctile(b, i)

    nc.compile()
    return nc


_NC_CACHE: dict = {}


def _get_nc(have_b, have_mask, loop_n=0):
    key = (have_b, have_mask, loop_n)
    if key not in _NC_CACHE:
        _NC_CACHE[key] = _build(have_b, have_mask, loop_n)
    return _NC_CACHE[key]


def _host_xt(xs):
    """xs: (NT, P, F) tile-major core shard -> pre-transposed layout where
    xt[i, p, k*128+t] = xs[i, t, k*128+p] (chunk-transposed for matmul lhsT)."""
    v = xs.reshape(NT, P, KC, P).transpose(0, 3, 2, 1)
    return np.ascontiguousarray(v).reshape(NT, P, F)


def make_core_maps(x, W, u, b=None, mask_f=None):
    """Build the 8 per-core input maps from full inputs."""
    x16 = x.astype(NPBF16)
    # W_host[p, k*F + f] = W[k*P + p, f]
    W_r = np.ascontiguousarray(
        W.reshape(KC, P, F).transpose(1, 0, 2).reshape(P, KC * F)).astype(NPBF16)
    u_r = np.ascontiguousarray(u.reshape(1, F)).astype(NPBF16)
    maps = []
    for core in range(NCORES):
        xs = np.ascontiguousarray(
            x16[core * B_LOC:(core + 1) * B_LOC].reshape(NT, P, F))
        m = {"x": xs, "xT": _host_xt(xs), "W": W_r, "u": u_r}
        if b is not None:
            m["b"] = np.ascontiguousarray(b.reshape(1, F)).astype(NPBF16)
        if mask_f is not None:
            ms = mask_f[core * B_LOC:(core + 1) * B_LOC]
            m["m"] = np.ascontiguousarray(
                ms.reshape(B_LOC, NTB, P).transpose(0, 2, 1))
        maps.append(m)
    return maps


def kernel(x, mask, W, b, u):
    x = np.asarray(x, dtype=np.float32)
    W = np.asarray(W, dtype=np.float32)
    b = np.asarray(b, dtype=np.float32)
    u = np.asarray(u, dtype=np.float32)
    mask_f = np.asarray(mask).astype(np.float32)

    have_b = bool(np.any(b != 0.0))
    have_mask = bool(np.any(mask_f != 1.0))

    nc = _get_nc(have_b, have_mask)
    in_maps = make_core_maps(x, W, u,
                             b if have_b else None,
                             mask_f if have_mask else None)
    res = run_bass_kernel_spmd(nc, in_maps, core_ids=list(range(NCORES)))
    out = np.stack([np.asarray(r["c"]).astype(np.float32).reshape(B_LOC, T, F)
                    for r in res.results])
    return out.reshape(B, T, F)


# revision 60
# speedup vs baseline: 2.3826x; 1.0032x over previous
"""Trainium2 Bass kernel for nn_Attention (cumulative masked softmax attention).

Reference computation:
    v   = tanh(x @ W + b)                  (B, T, F)
    a   = v . u                            (B, T)   -- query-independent logits
    e   = exp(a)[:, None, :] * tril * mask (B, T, T)
    alf = e / (sum_s e + EPS)
    c   = alf @ x                          (B, T, F)

Because the logits are query-independent and the mask is lower-triangular,
the (B,T,T) softmax-matmul collapses to a running weighted average:
    w[s]  = exp(a[s]) * mask[s]
    c[t]  = cumsum_s(w * x)[t] / (cumsum_s(w)[t] + EPS)
which is O(B*T*F) instead of O(B*T^2*F).

Sharding: data-parallel over batch B across 8 NeuronCores (2 batches/core).

v3 design (bf16):
  - All HBM traffic in bf16 (x, xT, W, u, c out) -- rel-err budget is 2e-2,
    bf16 keeps it ~2e-3 while halving DMA bytes.
  - The weights w fold into the 128x128 triangular matrix (tri_w = tri * w
    per tile, a cheap [128,128] gpsimd scale) instead of scaling x.
  - Cross-tile prefix offsets via 7 "step-mask" matmuls per batch (lhsT
    column m gets w[:,j] iff m > j) writing the per-tile offset rows [8, F]
    in PSUM; one copy to SBUF, then one [8,128]-selector matmul per tile
    broadcasts its offset row onto the tile prefix.
  - Denominator Z = prefix(w) via two tiny [P,8] matmuls + a free-dim scan
    (f32), reciprocal folded into the PSUM->SBUF readout scale.
  - Scheduling: xT loads split across the SP and ACT DMA queues ahead of
    the x loads; exp/tri-scales issued per pair so phase C of batch 0 can
    fill the tensor engine while batch 1's logits chain completes.
"""

import contextlib

import numpy as np
import ml_dtypes

import concourse.bass as bass  # noqa: F401
import concourse.tile as tile
from concourse import bacc, mybir
from concourse.bass_utils import run_bass_kernel_spmd

B, T, F = 16, 1024, 512
EPS = 1e-7
NCORES = 8
B_LOC = B // NCORES          # batches per core
R = B_LOC * T                # rows per core
P = 128                      # partition tile
NT = R // P                  # row tiles per core
NTB = T // P                 # row tiles per batch
NPAIR = NTB // 2             # tile pairs per batch
KC = F // P                  # contraction chunks

F32 = mybir.dt.float32
F32R = mybir.dt.float32r
BF16 = mybir.dt.bfloat16
NPBF16 = ml_dtypes.bfloat16


def _build(have_b: bool, have_mask: bool, loop_n: int = 0):
    """Build the per-core Bass module. loop_n > 0 wraps the body in a
    hardware For_i loop (used only for timing)."""
    nc = bacc.Bacc("TRN2", target_bir_lowering=False, debug=False)

    x_d = nc.dram_tensor("x", [NT, P, F], BF16, kind="ExternalInput")
    xt_d = nc.dram_tensor("xT", [NT, P, F], BF16, kind="ExternalInput")
    # W pre-arranged on host as (P, KC*F): W_host[p, k*F+f] = W[k*P+p, f]
    w_d = nc.dram_tensor("W", [P, KC * F], BF16, kind="ExternalInput")
    u_d = nc.dram_tensor("u", [1, F], BF16, kind="ExternalInput")
    if have_b:
        b_d = nc.dram_tensor("b", [1, F], BF16, kind="ExternalInput")
    if have_mask:
        m_d = nc.dram_tensor("m", [B_LOC, P, NTB], F32, kind="ExternalInput")
    c_d = nc.dram_tensor("c", [NT, P, F], BF16, kind="ExternalOutput")

    Tanh = mybir.ActivationFunctionType.Tanh
    Exp = mybir.ActivationFunctionType.Exp
    Copy = mybir.ActivationFunctionType.Copy
    ADD = mybir.AluOpType.add
    SUB = mybir.AluOpType.subtract

    with tile.TileContext(nc) as tc:
        with (
            tc.tile_pool(name="const", bufs=1) as const,
            tc.tile_pool(name="xp", bufs=2 * NPAIR) as xp,
            tc.tile_pool(name="xtp", bufs=2 * NPAIR) as xtp,
            tc.tile_pool(name="vp", bufs=2) as vp,
            tc.tile_pool(name="scrp", bufs=2) as scrp,
            tc.tile_pool(name="foldp", bufs=2) as foldp,
            tc.tile_pool(name="wp", bufs=2) as wp,
            tc.tile_pool(name="trwp", bufs=10) as trwp,
            tc.tile_pool(name="lwp", bufs=8) as lwp,
            tc.tile_pool(name="cumbp", bufs=2) as cumbp,
            tc.tile_pool(name="cp", bufs=8) as cp,
            tc.tile_pool(name="ps_v", bufs=2, space="PSUM") as ps_v_pool,
            tc.tile_pool(name="ps_P", bufs=4, space="PSUM") as ps_P_pool,
        ):
            # ---- constants ----
            # W split into per-chunk DMAs on the gpsimd queue so the first
            # matmul only waits on chunk 0 (+ the first xT tile) and the SP
            # queue can issue the xT loads immediately.
            W_sb = const.tile([P, KC, F], BF16)
            wr_ap = w_d.ap().rearrange("p (k f) -> p k f", k=KC)
            nc.scalar.dma_start(out=W_sb, in_=wr_ap)
            u_bc2 = const.tile([P, 2, F], BF16)
            nc.gpsimd.dma_start(out=u_bc2[:, 0, :],
                                in_=u_d.ap().to_broadcast((P, F)))
            nc.gpsimd.dma_start(out=u_bc2[:, 1, :],
                                in_=u_d.ap().to_broadcast((P, F)))
            onesf = const.tile([P, P], F32)
            nc.vector.memset(onesf, 1.0)
            ones_r = const.tile([P, P], F32R)
            nc.vector.tensor_copy(ones_r, onesf)

            if have_b:
                b_sb = const.tile([1, F], BF16)
                nc.sync.dma_start(out=b_sb, in_=b_d.ap())
                ones_row = const.tile([1, P], BF16)
                nc.vector.memset(ones_row, 1.0)

            # triangular matrices: triu[p, m] = 1 iff p <= m (inclusive prefix)
            triu_f = const.tile([P, P], F32)
            nc.gpsimd.memset(triu_f, 0.0)
            nc.gpsimd.affine_select(
                out=triu_f, in_=triu_f, compare_op=mybir.AluOpType.is_gt,
                fill=1.0, base=0, pattern=[[-1, P]], channel_multiplier=1)
            tri_bf = const.tile([P, P], BF16)
            nc.vector.tensor_copy(tri_bf, triu_f)
            triu_r = const.tile([P, P], F32R)
            nc.vector.tensor_copy(triu_r, triu_f)
            zeros8 = const.tile([P, NTB], F32)
            nc.vector.memset(zeros8, 0.0)
            # step masks: sm[j][p, m] = 1 iff m > j (offset matmul lhsT)
            sm_f = const.tile([P, NTB - 1, NTB], F32)
            sm_bf = const.tile([P, NTB - 1, NTB], BF16)
            nc.gpsimd.memset(sm_f, 1.0)
            for j in range(NTB - 1):
                nc.gpsimd.affine_select(
                    out=sm_f[:, j, :], in_=sm_f[:, j, :],
                    compare_op=mybir.AluOpType.is_gt,
                    fill=0.0, base=-j, pattern=[[1, NTB]], channel_multiplier=0)
            nc.vector.tensor_copy(sm_bf, sm_f)
            # row selectors: sel8[p, i*P+m] = 1 iff p == i (broadcast matmuls)
            sel_f = const.tile([NTB, NTB * P], F32)
            sel_bf = const.tile([NTB, NTB * P], BF16)
            nc.gpsimd.memset(sel_f, 1.0)
            nc.gpsimd.affine_select(
                out=sel_f, in_=sel_f, compare_op=mybir.AluOpType.is_equal,
                fill=0.0, base=0, pattern=[[-1, NTB], [0, P]],
                channel_multiplier=1)
            nc.vector.tensor_copy(sel_bf, sel_f)

            # ramp the PE clock gate while the first DMAs are in flight
            ps_warm = ps_P_pool.tile([P, P], F32, tag="pf", name="ps_warm")
            NWARM = 10
            for n in range(NWARM):
                nc.tensor.matmul(ps_warm, ones_r, ones_r,
                                 start=(n == 0), stop=(n == NWARM - 1))

            loop_ctx = (tc.For_i(0, loop_n, 1) if loop_n
                        else contextlib.nullcontext())
            with loop_ctx:
                # ---- all input DMAs up front: xT (needed first) on both
                # HWDGE queues, then x (needed in phase C) ----
                xts, xpairs = [], []
                for q in range(B_LOC * NPAIR):
                    i0 = 2 * q
                    # the last two pairs ride the ACT queue: their transfers
                    # finish before ACT's first tanh input is even ready, so
                    # they never block ACT compute but double the load cadence
                    eng = nc.scalar if q in (1, 6) else nc.sync
                    xT2 = xtp.tile([P, 2, F], BF16, name="xT2")
                    if q == 0:
                        nc.sync.dma_start(out=xT2[:, 0, :], in_=xt_d.ap()[0])
                        nc.sync.dma_start(out=xT2[:, 1, :], in_=xt_d.ap()[1])
                    else:
                        eng.dma_start(
                            out=xT2,
                            in_=xt_d.ap()[i0:i0 + 2].rearrange("j p f -> p j f"))
                    xts.append(xT2)
                for q in range(B_LOC * NPAIR):
                    i0 = 2 * q
                    x2 = xp.tile([P, 2, F], BF16, name="x2")
                    nc.gpsimd.dma_start(
                        out=x2,
                        in_=x_d.ap()[i0:i0 + 2].rearrange("j p f -> p j f"))
                    xpairs.append(x2)

                w_all_b, rec_b, lws_b, trws_b = [], [], [], []
                cumB_b = [None, None]
                c2_b = [None, None]
                m_all_b = []
                if have_mask:
                    for b in range(B_LOC):
                        m_all = wp.tile([P, NTB], F32, tag="m_all",
                                        name="m_all")
                        nc.sync.dma_start(out=m_all, in_=m_d.ap()[b])
                        m_all_b.append(m_all)

                def emit_phase_a(b):
                    """logits for batch b: per pair matmul->tanh->mul->fold->
                    reduce->exp, with tri/step scales issued per pair."""
                    alpha = wp.tile([P, NTB], BF16, tag="alpha", name="alpha")
                    w_all = wp.tile([P, NTB], F32, tag="w_all", name="w_all")
                    w_all_b.append(w_all)
                    lws, trws = [], []
                    lws_b.append(lws)
                    trws_b.append(trws)
                    for pp in range(NPAIR):
                        ps_v2 = ps_v_pool.tile([P, 2, F], F32, name="ps_v2")
                        for j in range(2):
                            t = NTB * b + 2 * pp + j
                            xT2 = xts[t // 2]
                            for k in range(KC):
                                nc.tensor.matmul(
                                    ps_v2[:, j, :],
                                    xT2[:, t % 2, k * P:(k + 1) * P],
                                    W_sb[:, k, :],
                                    start=(k == 0),
                                    stop=(k == KC - 1 and not have_b),
                                )
                            if have_b:
                                nc.tensor.matmul(ps_v2[:, j, :], ones_row,
                                                 b_sb, start=False, stop=True)
                        v2 = vp.tile([P, 2, F], BF16, name="v2")
                        nc.scalar.activation(out=v2, in_=ps_v2, func=Tanh)
                        scr2 = scrp.tile([P, 2, F], BF16, name="scr2")
                        nc.vector.tensor_mul(scr2, v2, u_bc2)
                        fld = foldp.tile([P, 2, F // 2], BF16, name="fld")
                        nc.vector.tensor_tensor(
                            out=fld, in0=scr2[:, :, 0:F // 2],
                            in1=scr2[:, :, F // 2:F], op=ADD)
                        sl = slice(2 * pp, 2 * pp + 2)
                        with nc.allow_low_precision(
                                reason="bf16 logits; 2e-2 rel-err budget"):
                            nc.vector.tensor_reduce(
                                alpha[:, sl], fld,
                                axis=mybir.AxisListType.X, op=ADD)
                        nc.scalar.activation(out=w_all[:, sl],
                                             in_=alpha[:, sl], func=Exp)
                        if have_mask:
                            nc.vector.tensor_mul(w_all[:, sl], w_all[:, sl],
                                                 m_all_b[b][:, sl])
                        for i in (2 * pp, 2 * pp + 1):
                            if i < NTB - 1:
                                lw = lwp.tile([P, NTB], BF16, name="lw")
                                nc.gpsimd.tensor_scalar_mul(
                                    lw, sm_bf[:, i, :], w_all[:, i:i + 1])
                                lws.append(lw)
                            trw = trwp.tile([P, P], BF16, name="trw")
                            nc.gpsimd.tensor_scalar_mul(
                                trw, tri_bf, w_all[:, i:i + 1])
                            trws.append(trw)

                def emit_zrec(b):
                    w_r = wp.tile([P, NTB], F32R, tag="w_r", name="w_r")
                    nc.vector.tensor_copy(w_r, w_all_b[b])
                    ps_z = ps_P_pool.tile([P, 2 * NTB], F32, tag="pf",
                                          name="ps_z")
                    nc.tensor.matmul(ps_z[:, 0:NTB], triu_r, w_r,
                                     start=True, stop=True)
                    nc.tensor.matmul(ps_z[:, NTB:2 * NTB], ones_r, w_r,
                                     start=True, stop=True)
                    S = wp.tile([P, NTB], F32, tag="S", name="S")
                    nc.vector.tensor_tensor_scan(
                        out=S, data0=ps_z[:, NTB:2 * NTB], data1=zeros8,
                        initial=0.0, op0=ADD, op1=ADD)
                    Z = wp.tile([P, NTB], F32, tag="Z", name="Z")
                    nc.vector.tensor_tensor(out=Z, in0=S,
                                            in1=ps_z[:, NTB:2 * NTB], op=SUB)
                    nc.vector.tensor_tensor(out=Z, in0=Z, in1=ps_z[:, 0:NTB],
                                            op=ADD)
                    nc.vector.tensor_scalar_add(Z, Z, EPS)
                    rec = wp.tile([P, NTB], F32, tag="rec", name="rec")
                    nc.vector.reciprocal(rec, Z)
                    rec_b.append(rec)

                def emit_offsets(b):
                    # ps_cum[m, f] = sum_{j<m} (w * x) tile totals for batch b
                    ps_cum = ps_P_pool.tile([NTB, F], F32, tag="pf",
                                            name="ps_cum")
                    for j in range(NTB - 1):
                        nc.tensor.matmul(ps_cum, lws_b[b][j],
                                         xpairs[NPAIR * b + j // 2][:, j % 2, :],
                                         start=(j == 0), stop=(j == NTB - 2))
                    cumB = cumbp.tile([NTB, F], BF16, name="cumB")
                    nc.scalar.activation(out=cumB, in_=ps_cum, func=Copy)
                    cumB_b[b] = cumB

                ro_pat = [0, 1] * 8
                ro_n = [0]

                def emit_ctile(b, i):
                    ps_P = ps_P_pool.tile([P, F], F32, tag="pf", name="ps_P")
                    nc.tensor.matmul(ps_P, trws_b[b][i],
                                     xpairs[NPAIR * b + i // 2][:, i % 2, :],
                                     start=True, stop=(i == 0))
                    if i > 0:
                        nc.tensor.matmul(ps_P, sel_bf[:, i * P:(i + 1) * P],
                                         cumB_b[b], start=False, stop=True)
                    rec = rec_b[b]
                    if i % 2 == 0:
                        c2_b[b] = cp.tile([P, 2, F], BF16, name="c2")
                    c_sl = c2_b[b][:, i % 2, :]
                    if ro_pat[ro_n[0]] == 0:
                        nc.scalar.activation(out=c_sl, in_=ps_P, func=Copy,
                                             scale=rec[:, i:i + 1])
                    else:
                        nc.vector.tensor_scalar_mul(c_sl, ps_P,
                                                    rec[:, i:i + 1])
                    ro_n[0] += 1
                    i0 = NTB * b + i
                    if b == 1 and i >= NTB - 2:
                        nc.sync.dma_start(out=c_d.ap()[i0], in_=c_sl)
                    elif i % 2 == 1:
                        nc.sync.dma_start(
                            out=c_d.ap()[i0 - 1:i0 + 1].rearrange(
                                "j p f -> p j f"),
                            in_=c2_b[b])

                emit_phase_a(0)
                emit_phase_a(1)
                emit_zrec(0)
                emit_offsets(0)
                for i in range(7):
                    emit_ctile(0, i)
                emit_zrec(1)
                emit_offsets(1)
                seq = [(0, 7), (1, 0), (1, 1), (1, 2), (1, 3),
                       (1, 4), (1, 5), (1, 6), (1, 7)]
                for b, i in seq:
                    emit_ctile(b, i)

    nc.compile()
    return nc


_NC_CACHE: dict = {}


def _get_nc(have_b, have_mask, loop_n=0):
    key = (have_b, have_mask, loop_n)
    if key not in _NC_CACHE:
        _NC_CACHE[key] = _build(have_b, have_mask, loop_n)
    return _NC_CACHE[key]


def _host_xt(xs):
    """xs: (NT, P, F) tile-major core shard -> pre-transposed layout where
    xt[i, p, k*128+t] = xs[i, t, k*128+p] (chunk-transposed for matmul lhsT)."""
    v = xs.reshape(NT, P, KC, P).transpose(0, 3, 2, 1)
    return np.ascontiguousarray(v).reshape(NT, P, F)


def make_core_maps(x, W, u, b=None, mask_f=None):
    """Build the 8 per-core input maps from full inputs."""
    x16 = x.astype(NPBF16)
    # W_host[p, k*F + f] = W[k*P + p, f]
    W_r = np.ascontiguousarray(
        W.reshape(KC, P, F).transpose(1, 0, 2).reshape(P, KC * F)).astype(NPBF16)
    u_r = np.ascontiguousarray(u.reshape(1, F)).astype(NPBF16)
    maps = []
    for core in range(NCORES):
        xs = np.ascontiguousarray(
            x16[core * B_LOC:(core + 1) * B_LOC].reshape(NT, P, F))
        m = {"x": xs, "xT": _host_xt(xs), "W": W_r, "u": u_r}
        if b is not None:
            m["b"] = np.ascontiguousarray(b.reshape(1, F)).astype(NPBF16)
        if mask_f is not None:
            ms = mask_f[core * B_LOC:(core + 1) * B_LOC]
            m["m"] = np.ascontiguousarray(
                ms.reshape(B_LOC, NTB, P).transpose(0, 2, 1))
        maps.append(m)
    return maps


def kernel(x, mask, W, b, u):
    x = np.asarray(x, dtype=np.float32)
    W = np.asarray(W, dtype=np.float32)
    b = np.asarray(b, dtype=np.float32)
    u = np.asarray(u, dtype=np.float32)
    mask_f = np.asarray(mask).astype(np.float32)

    have_b = bool(np.any(b != 0.0))
    have_mask = bool(np.any(mask_f != 1.0))

    nc = _get_nc(have_b, have_mask)
    in_maps = make_core_maps(x, W, u,
                             b if have_b else None,
                             mask_f if have_mask else None)
    res = run_bass_kernel_spmd(nc, in_maps, core_ids=list(range(NCORES)))
    out = np.stack([np.asarray(r["c"]).astype(np.float32).reshape(B_LOC, T, F)
                    for r in res.results])
    return out.reshape(B, T, F)


# revision 65
# speedup vs baseline: 2.4233x; 1.0171x over previous
"""Trainium2 Bass kernel for nn_Attention (cumulative masked softmax attention).

Reference computation:
    v   = tanh(x @ W + b)                  (B, T, F)
    a   = v . u                            (B, T)   -- query-independent logits
    e   = exp(a)[:, None, :] * tril * mask (B, T, T)
    alf = e / (sum_s e + EPS)
    c   = alf @ x                          (B, T, F)

Because the logits are query-independent and the mask is lower-triangular,
the (B,T,T) softmax-matmul collapses to a running weighted average:
    w[s]  = exp(a[s]) * mask[s]
    c[t]  = cumsum_s(w * x)[t] / (cumsum_s(w)[t] + EPS)
which is O(B*T*F) instead of O(B*T^2*F).

Sharding: data-parallel over batch B across 8 NeuronCores (2 batches/core).

v3 design (bf16):
  - All HBM traffic in bf16 (x, xT, W, u, c out) -- rel-err budget is 2e-2,
    bf16 keeps it ~2e-3 while halving DMA bytes.
  - The weights w fold into the 128x128 triangular matrix (tri_w = tri * w
    per tile, a cheap [128,128] gpsimd scale) instead of scaling x.
  - Cross-tile prefix offsets via 7 "step-mask" matmuls per batch (lhsT
    column m gets w[:,j] iff m > j) writing the per-tile offset rows [8, F]
    in PSUM; one copy to SBUF, then one [8,128]-selector matmul per tile
    broadcasts its offset row onto the tile prefix.
  - Denominator Z = prefix(w) via two tiny [P,8] matmuls + a free-dim scan
    (f32), reciprocal folded into the PSUM->SBUF readout scale.
  - Scheduling: xT loads split across the SP and ACT DMA queues ahead of
    the x loads; exp/tri-scales issued per pair so phase C of batch 0 can
    fill the tensor engine while batch 1's logits chain completes.
"""

import contextlib

import numpy as np
import ml_dtypes

import concourse.bass as bass  # noqa: F401
import concourse.tile as tile
from concourse import bacc, mybir
from concourse.bass_utils import run_bass_kernel_spmd

B, T, F = 16, 1024, 512
EPS = 1e-7
NCORES = 8
B_LOC = B // NCORES          # batches per core
R = B_LOC * T                # rows per core
P = 128                      # partition tile
NT = R // P                  # row tiles per core
NTB = T // P                 # row tiles per batch
NPAIR = NTB // 2             # tile pairs per batch
KC = F // P                  # contraction chunks

F32 = mybir.dt.float32
F32R = mybir.dt.float32r
BF16 = mybir.dt.bfloat16
NPBF16 = ml_dtypes.bfloat16


def _build(have_b: bool, have_mask: bool, loop_n: int = 0):
    """Build the per-core Bass module. loop_n > 0 wraps the body in a
    hardware For_i loop (used only for timing)."""
    nc = bacc.Bacc("TRN2", target_bir_lowering=False, debug=False)

    x_d = nc.dram_tensor("x", [NT, P, F], BF16, kind="ExternalInput")
    xt_d = nc.dram_tensor("xT", [NT, P, F], BF16, kind="ExternalInput")
    # W pre-arranged on host as (P, KC*F): W_host[p, k*F+f] = W[k*P+p, f]
    w_d = nc.dram_tensor("W", [P, KC * F], BF16, kind="ExternalInput")
    u_d = nc.dram_tensor("u", [1, F], BF16, kind="ExternalInput")
    if have_b:
        b_d = nc.dram_tensor("b", [1, F], BF16, kind="ExternalInput")
    if have_mask:
        m_d = nc.dram_tensor("m", [B_LOC, P, NTB], F32, kind="ExternalInput")
    c_d = nc.dram_tensor("c", [NT, P, F], BF16, kind="ExternalOutput")

    Tanh = mybir.ActivationFunctionType.Tanh
    Exp = mybir.ActivationFunctionType.Exp
    Copy = mybir.ActivationFunctionType.Copy
    ADD = mybir.AluOpType.add
    SUB = mybir.AluOpType.subtract

    with tile.TileContext(nc) as tc:
        with (
            tc.tile_pool(name="const", bufs=1) as const,
            tc.tile_pool(name="xp", bufs=2 * NPAIR) as xp,
            tc.tile_pool(name="xtp", bufs=2 * NPAIR) as xtp,
            tc.tile_pool(name="vp", bufs=2) as vp,
            tc.tile_pool(name="scrp", bufs=2) as scrp,
            tc.tile_pool(name="foldp", bufs=2) as foldp,
            tc.tile_pool(name="wp", bufs=2) as wp,
            tc.tile_pool(name="trwp", bufs=10) as trwp,
            tc.tile_pool(name="lwp", bufs=8) as lwp,
            tc.tile_pool(name="cumbp", bufs=2) as cumbp,
            tc.tile_pool(name="cp", bufs=8) as cp,
            tc.tile_pool(name="ps_v", bufs=2, space="PSUM") as ps_v_pool,
            tc.tile_pool(name="ps_P", bufs=4, space="PSUM") as ps_P_pool,
        ):
            # ---- constants ----
            # W split into per-chunk DMAs on the gpsimd queue so the first
            # matmul only waits on chunk 0 (+ the first xT tile) and the SP
            # queue can issue the xT loads immediately.
            W_sb = const.tile([P, KC, F], BF16)
            wr_ap = w_d.ap().rearrange("p (k f) -> p k f", k=KC)
            nc.scalar.dma_start(out=W_sb, in_=wr_ap)
            u_bc2 = const.tile([P, 2, F], BF16)
            nc.gpsimd.dma_start(out=u_bc2[:, 0, :],
                                in_=u_d.ap().to_broadcast((P, F)))
            nc.gpsimd.dma_start(out=u_bc2[:, 1, :],
                                in_=u_d.ap().to_broadcast((P, F)))
            onesf = const.tile([P, P], F32)
            nc.vector.memset(onesf, 1.0)
            ones_r = const.tile([P, P], F32R)
            nc.vector.tensor_copy(ones_r, onesf)

            if have_b:
                b_sb = const.tile([1, F], BF16)
                nc.sync.dma_start(out=b_sb, in_=b_d.ap())
                ones_row = const.tile([1, P], BF16)
                nc.vector.memset(ones_row, 1.0)

            # triangular matrices: triu[p, m] = 1 iff p <= m (inclusive prefix)
            triu_f = const.tile([P, P], F32)
            nc.gpsimd.memset(triu_f, 0.0)
            nc.gpsimd.affine_select(
                out=triu_f, in_=triu_f, compare_op=mybir.AluOpType.is_gt,
                fill=1.0, base=0, pattern=[[-1, P]], channel_multiplier=1)
            tri_bf = const.tile([P, P], BF16)
            nc.vector.tensor_copy(tri_bf, triu_f)
            triu_r = const.tile([P, P], F32R)
            nc.vector.tensor_copy(triu_r, triu_f)
            zeros8 = const.tile([P, NTB], F32)
            nc.vector.memset(zeros8, 0.0)
            # step masks: sm[j][p, m] = 1 iff m > j (offset matmul lhsT)
            sm_f = const.tile([P, NTB - 1, NTB], F32)
            sm_bf = const.tile([P, NTB - 1, NTB], BF16)
            nc.gpsimd.memset(sm_f, 1.0)
            for j in range(NTB - 1):
                nc.gpsimd.affine_select(
                    out=sm_f[:, j, :], in_=sm_f[:, j, :],
                    compare_op=mybir.AluOpType.is_gt,
                    fill=0.0, base=-j, pattern=[[1, NTB]], channel_multiplier=0)
            nc.vector.tensor_copy(sm_bf, sm_f)
            # row selectors: sel8[p, i*P+m] = 1 iff p == i (broadcast matmuls)
            sel_f = const.tile([NTB, NTB * P], F32)
            sel_bf = const.tile([NTB, NTB * P], BF16)
            nc.gpsimd.memset(sel_f, 1.0)
            nc.gpsimd.affine_select(
                out=sel_f, in_=sel_f, compare_op=mybir.AluOpType.is_equal,
                fill=0.0, base=0, pattern=[[-1, NTB], [0, P]],
                channel_multiplier=1)
            nc.vector.tensor_copy(sel_bf, sel_f)

            # ramp the PE clock gate while the first DMAs are in flight
            ps_warm = ps_P_pool.tile([P, P], F32, tag="pf", name="ps_warm")
            NWARM = 14
            for n in range(NWARM):
                nc.tensor.matmul(ps_warm, ones_r, ones_r,
                                 start=(n == 0), stop=(n == NWARM - 1))

            loop_ctx = (tc.For_i(0, loop_n, 1) if loop_n
                        else contextlib.nullcontext())
            with loop_ctx:
                # ---- all input DMAs up front: xT (needed first) on both
                # HWDGE queues, then x (needed in phase C) ----
                xts, xpairs = [], []
                for q in range(B_LOC * NPAIR):
                    i0 = 2 * q
                    # the last two pairs ride the ACT queue: their transfers
                    # finish before ACT's first tanh input is even ready, so
                    # they never block ACT compute but double the load cadence
                    eng = nc.scalar if q in (1, 6) else nc.sync
                    xT2 = xtp.tile([P, 2, F], BF16, name="xT2")
                    if q == 0:
                        nc.sync.dma_start(out=xT2[:, 0, :], in_=xt_d.ap()[0])
                        nc.sync.dma_start(out=xT2[:, 1, :], in_=xt_d.ap()[1])
                    else:
                        eng.dma_start(
                            out=xT2,
                            in_=xt_d.ap()[i0:i0 + 2].rearrange("j p f -> p j f"))
                    xts.append(xT2)
                for q in range(B_LOC * NPAIR):
                    i0 = 2 * q
                    x2 = xp.tile([P, 2, F], BF16, name="x2")
                    nc.gpsimd.dma_start(
                        out=x2,
                        in_=x_d.ap()[i0:i0 + 2].rearrange("j p f -> p j f"))
                    xpairs.append(x2)

                w_all_b, rec_b, lws_b, trws_b = [], [], [], []
                cumB_b = [None, None]
                c2_b = [None, None]
                m_all_b = []
                if have_mask:
                    for b in range(B_LOC):
                        m_all = wp.tile([P, NTB], F32, tag="m_all",
                                        name="m_all")
                        nc.sync.dma_start(out=m_all, in_=m_d.ap()[b])
                        m_all_b.append(m_all)

                def emit_phase_a(b):
                    """logits for batch b: per pair matmul->tanh->mul->fold->
                    reduce->exp, with tri/step scales issued per pair."""
                    alpha = wp.tile([P, NTB], BF16, tag="alpha", name="alpha")
                    w_all = wp.tile([P, NTB], F32, tag="w_all", name="w_all")
                    w_all_b.append(w_all)
                    lws, trws = [], []
                    lws_b.append(lws)
                    trws_b.append(trws)
                    for pp in range(NPAIR):
                        ps_v2 = ps_v_pool.tile([P, 2, F], F32, name="ps_v2")
                        for j in range(2):
                            t = NTB * b + 2 * pp + j
                            xT2 = xts[t // 2]
                            for k in range(KC):
                                nc.tensor.matmul(
                                    ps_v2[:, j, :],
                                    xT2[:, t % 2, k * P:(k + 1) * P],
                                    W_sb[:, k, :],
                                    start=(k == 0),
                                    stop=(k == KC - 1 and not have_b),
                                )
                            if have_b:
                                nc.tensor.matmul(ps_v2[:, j, :], ones_row,
                                                 b_sb, start=False, stop=True)
                        v2 = vp.tile([P, 2, F], BF16, name="v2")
                        nc.scalar.activation(out=v2, in_=ps_v2, func=Tanh)
                        scr2 = scrp.tile([P, 2, F], BF16, name="scr2")
                        nc.vector.tensor_mul(scr2, v2, u_bc2)
                        fld = foldp.tile([P, 2, F // 2], BF16, name="fld")
                        nc.vector.tensor_tensor(
                            out=fld, in0=scr2[:, :, 0:F // 2],
                            in1=scr2[:, :, F // 2:F], op=ADD)
                        sl = slice(2 * pp, 2 * pp + 2)
                        with nc.allow_low_precision(
                                reason="bf16 logits; 2e-2 rel-err budget"):
                            nc.vector.tensor_reduce(
                                alpha[:, sl], fld,
                                axis=mybir.AxisListType.X, op=ADD)
                        nc.scalar.activation(out=w_all[:, sl],
                                             in_=alpha[:, sl], func=Exp)
                        if have_mask:
                            nc.vector.tensor_mul(w_all[:, sl], w_all[:, sl],
                                                 m_all_b[b][:, sl])
                        for i in (2 * pp, 2 * pp + 1):
                            if i < NTB - 1:
                                lw = lwp.tile([P, NTB], BF16, name="lw")
                                nc.gpsimd.tensor_scalar_mul(
                                    lw, sm_bf[:, i, :], w_all[:, i:i + 1])
                                lws.append(lw)
                            trw = trwp.tile([P, P], BF16, name="trw")
                            nc.gpsimd.tensor_scalar_mul(
                                trw, tri_bf, w_all[:, i:i + 1])
                            trws.append(trw)

                def emit_zrec(b):
                    # f32r copy on ACT: keeps the congested DVE FIFO clear
                    w_r = wp.tile([P, NTB], F32R, tag="w_r", name="w_r")
                    nc.scalar.activation(out=w_r, in_=w_all_b[b], func=Copy)
                    ps_z = ps_P_pool.tile([P, 2 * NTB], F32, tag="pf",
                                          name="ps_z")
                    nc.tensor.matmul(ps_z[:, 0:NTB], triu_r, w_r,
                                     start=True, stop=True)
                    nc.tensor.matmul(ps_z[:, NTB:2 * NTB], ones_r, w_r,
                                     start=True, stop=True)
                    S = wp.tile([P, NTB], F32, tag="S", name="S")
                    nc.vector.tensor_tensor_scan(
                        out=S, data0=ps_z[:, NTB:2 * NTB], data1=zeros8,
                        initial=0.0, op0=ADD, op1=ADD)
                    Z = wp.tile([P, NTB], F32, tag="Z", name="Z")
                    nc.vector.tensor_tensor(out=Z, in0=S,
                                            in1=ps_z[:, NTB:2 * NTB], op=SUB)
                    nc.vector.tensor_tensor(out=Z, in0=Z, in1=ps_z[:, 0:NTB],
                                            op=ADD)
                    if have_mask:
                        # only a mask can zero the denominator; without one
                        # Z >= exp(alpha) > 0 and the EPS add is dead weight
                        nc.vector.tensor_scalar_add(Z, Z, EPS)
                    rec = wp.tile([P, NTB], F32, tag="rec", name="rec")
                    nc.vector.reciprocal(rec, Z)
                    rec_b.append(rec)

                def emit_offsets(b):
                    # ps_cum[m, f] = sum_{j<m} (w * x) tile totals for batch b
                    ps_cum = ps_P_pool.tile([NTB, F], F32, tag="pf",
                                            name="ps_cum")
                    for j in range(NTB - 1):
                        nc.tensor.matmul(ps_cum, lws_b[b][j],
                                         xpairs[NPAIR * b + j // 2][:, j % 2, :],
                                         start=(j == 0), stop=(j == NTB - 2))
                    cumB = cumbp.tile([NTB, F], BF16, name="cumB")
                    nc.scalar.activation(out=cumB, in_=ps_cum, func=Copy)
                    cumB_b[b] = cumB

                ro_pat = [0, 1, 0, 1, 0, 1, 1, 0, 1, 0, 1, 0, 1, 0, 1, 0]
                ro_n = [0]

                def emit_ctile(b, i):
                    ps_P = ps_P_pool.tile([P, F], F32, tag="pf", name="ps_P")
                    nc.tensor.matmul(ps_P, trws_b[b][i],
                                     xpairs[NPAIR * b + i // 2][:, i % 2, :],
                                     start=True, stop=(i == 0))
                    if i > 0:
                        nc.tensor.matmul(ps_P, sel_bf[:, i * P:(i + 1) * P],
                                         cumB_b[b], start=False, stop=True)
                    rec = rec_b[b]
                    if i % 2 == 0:
                        c2_b[b] = cp.tile([P, 2, F], BF16, name="c2")
                    c_sl = c2_b[b][:, i % 2, :]
                    if ro_pat[ro_n[0]] == 0:
                        nc.scalar.activation(out=c_sl, in_=ps_P, func=Copy,
                                             scale=rec[:, i:i + 1])
                    else:
                        nc.vector.tensor_scalar_mul(c_sl, ps_P,
                                                    rec[:, i:i + 1])
                    ro_n[0] += 1
                    i0 = NTB * b + i
                    if i % 2 == 1:
                        nc.sync.dma_start(
                            out=c_d.ap()[i0 - 1:i0 + 1].rearrange(
                                "j p f -> p j f"),
                            in_=c2_b[b])

                emit_phase_a(0)
                emit_phase_a(1)
                emit_zrec(0)
                emit_offsets(0)
                for i in range(7):
                    emit_ctile(0, i)
                emit_zrec(1)
                emit_offsets(1)
                seq = [(0, 7), (1, 0), (1, 1), (1, 2), (1, 3),
                       (1, 4), (1, 5), (1, 6), (1, 7)]
                for b, i in seq:
                    emit_ctile(b, i)

    nc.compile()
    return nc


_NC_CACHE: dict = {}


def _get_nc(have_b, have_mask, loop_n=0):
    key = (have_b, have_mask, loop_n)
    if key not in _NC_CACHE:
        _NC_CACHE[key] = _build(have_b, have_mask, loop_n)
    return _NC_CACHE[key]


def _host_xt(xs):
    """xs: (NT, P, F) tile-major core shard -> pre-transposed layout where
    xt[i, p, k*128+t] = xs[i, t, k*128+p] (chunk-transposed for matmul lhsT)."""
    v = xs.reshape(NT, P, KC, P).transpose(0, 3, 2, 1)
    return np.ascontiguousarray(v).reshape(NT, P, F)


def make_core_maps(x, W, u, b=None, mask_f=None):
    """Build the 8 per-core input maps from full inputs."""
    x16 = x.astype(NPBF16)
    # W_host[p, k*F + f] = W[k*P + p, f]
    W_r = np.ascontiguousarray(
        W.reshape(KC, P, F).transpose(1, 0, 2).reshape(P, KC * F)).astype(NPBF16)
    u_r = np.ascontiguousarray(u.reshape(1, F)).astype(NPBF16)
    maps = []
    for core in range(NCORES):
        xs = np.ascontiguousarray(
            x16[core * B_LOC:(core + 1) * B_LOC].reshape(NT, P, F))
        m = {"x": xs, "xT": _host_xt(xs), "W": W_r, "u": u_r}
        if b is not None:
            m["b"] = np.ascontiguousarray(b.reshape(1, F)).astype(NPBF16)
        if mask_f is not None:
            ms = mask_f[core * B_LOC:(core + 1) * B_LOC]
            m["m"] = np.ascontiguousarray(
                ms.reshape(B_LOC, NTB, P).transpose(0, 2, 1))
        maps.append(m)
    return maps


def kernel(x, mask, W, b, u):
    x = np.asarray(x, dtype=np.float32)
    W = np.asarray(W, dtype=np.float32)
    b = np.asarray(b, dtype=np.float32)
    u = np.asarray(u, dtype=np.float32)
    mask_f = np.asarray(mask).astype(np.float32)

    have_b = bool(np.any(b != 0.0))
    have_mask = bool(np.any(mask_f != 1.0))

    nc = _get_nc(have_b, have_mask)
    in_maps = make_core_maps(x, W, u,
                             b if have_b else None,
                             mask_f if have_mask else None)
    res = run_bass_kernel_spmd(nc, in_maps, core_ids=list(range(NCORES)))
    out = np.stack([np.asarray(r["c"]).astype(np.float32).reshape(B_LOC, T, F)
                    for r in res.results])
    return out.reshape(B, T, F)


# revision 68
# speedup vs baseline: 2.7310x; 1.1270x over previous
"""Trainium2 Bass kernel for nn_Attention (cumulative masked softmax attention).

Reference computation:
    v   = tanh(x @ W + b)                  (B, T, F)
    a   = v . u                            (B, T)   -- query-independent logits
    e   = exp(a)[:, None, :] * tril * mask (B, T, T)
    alf = e / (sum_s e + EPS)
    c   = alf @ x                          (B, T, F)

Because the logits are query-independent and the mask is lower-triangular,
the (B,T,T) softmax-matmul collapses to a running weighted average:
    w[s]  = exp(a[s]) * mask[s]
    c[t]  = cumsum_s(w * x)[t] / (cumsum_s(w)[t] + EPS)
which is O(B*T*F) instead of O(B*T^2*F).

Sharding: data-parallel over batch B across 8 NeuronCores (2 batches/core).

v3 design (bf16):
  - All HBM traffic in bf16 (x, xT, W, u, c out) -- rel-err budget is 2e-2,
    bf16 keeps it ~2e-3 while halving DMA bytes.
  - The weights w fold into the 128x128 triangular matrix (tri_w = tri * w
    per tile, a cheap [128,128] gpsimd scale) instead of scaling x.
  - Cross-tile prefix offsets via 7 "step-mask" matmuls per batch (lhsT
    column m gets w[:,j] iff m > j) writing the per-tile offset rows [8, F]
    in PSUM; one copy to SBUF, then one [8,128]-selector matmul per tile
    broadcasts its offset row onto the tile prefix.
  - Denominator Z = prefix(w) via two tiny [P,8] matmuls + a free-dim scan
    (f32), reciprocal folded into the PSUM->SBUF readout scale.
  - Scheduling: xT loads split across the SP and ACT DMA queues ahead of
    the x loads; exp/tri-scales issued per pair so phase C of batch 0 can
    fill the tensor engine while batch 1's logits chain completes.
"""

import contextlib

import numpy as np
import ml_dtypes

import concourse.bass as bass  # noqa: F401
import concourse.tile as tile
from concourse import bacc, mybir
from concourse.bass_utils import run_bass_kernel_spmd

B, T, F = 16, 1024, 512
EPS = 1e-7
NCORES = 8
B_LOC = B // NCORES          # batches per core
R = B_LOC * T                # rows per core
P = 128                      # partition tile
NT = R // P                  # row tiles per core
NTB = T // P                 # row tiles per batch
NPAIR = NTB // 2             # tile pairs per batch
KC = F // P                  # contraction chunks

F32 = mybir.dt.float32
F32R = mybir.dt.float32r
BF16 = mybir.dt.bfloat16
NPBF16 = ml_dtypes.bfloat16


def _build(have_b: bool, have_mask: bool, loop_n: int = 0):
    """Build the per-core Bass module. loop_n > 0 wraps the body in a
    hardware For_i loop (used only for timing)."""
    nc = bacc.Bacc("TRN2", target_bir_lowering=False, debug=False)

    x_d = nc.dram_tensor("x", [NT, P, F], BF16, kind="ExternalInput")
    xt_d = nc.dram_tensor("xT", [NT, P, F], BF16, kind="ExternalInput")
    # W pre-arranged on host as (P, KC*F): W_host[p, k*F+f] = W[k*P+p, f]
    w_d = nc.dram_tensor("W", [P, KC * F], BF16, kind="ExternalInput")
    u_d = nc.dram_tensor("u", [1, F], BF16, kind="ExternalInput")
    if have_b:
        b_d = nc.dram_tensor("b", [1, F], BF16, kind="ExternalInput")
    if have_mask:
        m_d = nc.dram_tensor("m", [B_LOC, P, NTB], F32, kind="ExternalInput")
    c_d = nc.dram_tensor("c", [NT, P, F], BF16, kind="ExternalOutput")

    Tanh = mybir.ActivationFunctionType.Tanh
    Exp = mybir.ActivationFunctionType.Exp
    Copy = mybir.ActivationFunctionType.Copy
    ADD = mybir.AluOpType.add
    SUB = mybir.AluOpType.subtract

    with tile.TileContext(nc) as tc:
        with (
            tc.tile_pool(name="const", bufs=1) as const,
            tc.tile_pool(name="xp", bufs=2 * NPAIR) as xp,
            tc.tile_pool(name="xtp", bufs=2 * NPAIR) as xtp,
            tc.tile_pool(name="vp", bufs=2) as vp,
            tc.tile_pool(name="scrp", bufs=2) as scrp,
            tc.tile_pool(name="foldp", bufs=2) as foldp,
            tc.tile_pool(name="wp", bufs=2) as wp,
            tc.tile_pool(name="trwp", bufs=10) as trwp,
            tc.tile_pool(name="lwp", bufs=8) as lwp,
            tc.tile_pool(name="cumbp", bufs=2) as cumbp,
            tc.tile_pool(name="cp", bufs=8) as cp,
            tc.tile_pool(name="ps_v", bufs=2, space="PSUM") as ps_v_pool,
            tc.tile_pool(name="ps_P", bufs=4, space="PSUM") as ps_P_pool,
        ):
            # ---- constants ----
            # W split into per-chunk DMAs on the gpsimd queue so the first
            # matmul only waits on chunk 0 (+ the first xT tile) and the SP
            # queue can issue the xT loads immediately.
            W_sb = const.tile([P, KC, F], BF16)
            wr_ap = w_d.ap().rearrange("p (k f) -> p k f", k=KC)
            nc.scalar.dma_start(out=W_sb, in_=wr_ap)
            u_bc2 = const.tile([P, 2, F], BF16)
            nc.gpsimd.dma_start(out=u_bc2[:, 0, :],
                                in_=u_d.ap().to_broadcast((P, F)))
            nc.gpsimd.dma_start(out=u_bc2[:, 1, :],
                                in_=u_d.ap().to_broadcast((P, F)))
            onesf = const.tile([P, P], F32)
            nc.vector.memset(onesf, 1.0)
            ones_r = const.tile([P, P], F32R)
            nc.vector.tensor_copy(ones_r, onesf)

            if have_b:
                b_sb = const.tile([1, F], BF16)
                nc.sync.dma_start(out=b_sb, in_=b_d.ap())
                ones_row = const.tile([1, P], BF16)
                nc.vector.memset(ones_row, 1.0)

            # triangular matrices: triu[p, m] = 1 iff p <= m (inclusive prefix)
            triu_f = const.tile([P, P], F32)
            nc.gpsimd.memset(triu_f, 0.0)
            nc.gpsimd.affine_select(
                out=triu_f, in_=triu_f, compare_op=mybir.AluOpType.is_gt,
                fill=1.0, base=0, pattern=[[-1, P]], channel_multiplier=1)
            tri_bf = const.tile([P, P], BF16)
            nc.vector.tensor_copy(tri_bf, triu_f)
            triu_r = const.tile([P, P], F32R)
            nc.vector.tensor_copy(triu_r, triu_f)
            zeros8 = const.tile([P, NTB], F32)
            nc.vector.memset(zeros8, 0.0)
            # step masks: sm[j][p, m] = 1 iff m > j (offset matmul lhsT)
            sm_f = const.tile([P, NTB - 1, NTB], F32)
            sm_bf = const.tile([P, NTB - 1, NTB], BF16)
            nc.gpsimd.memset(sm_f, 1.0)
            for j in range(NTB - 1):
                nc.gpsimd.affine_select(
                    out=sm_f[:, j, :], in_=sm_f[:, j, :],
                    compare_op=mybir.AluOpType.is_gt,
                    fill=0.0, base=-j, pattern=[[1, NTB]], channel_multiplier=0)
            nc.vector.tensor_copy(sm_bf, sm_f)
            # row selectors: sel8[p, i*P+m] = 1 iff p == i (broadcast matmuls)
            sel_f = const.tile([NTB, NTB * P], F32)
            sel_bf = const.tile([NTB, NTB * P], BF16)
            nc.gpsimd.memset(sel_f, 1.0)
            nc.gpsimd.affine_select(
                out=sel_f, in_=sel_f, compare_op=mybir.AluOpType.is_equal,
                fill=0.0, base=0, pattern=[[-1, NTB], [0, P]],
                channel_multiplier=1)
            nc.vector.tensor_copy(sel_bf, sel_f)

            # ramp the PE clock gate while the first DMAs are in flight
            ps_warm = ps_P_pool.tile([P, P], F32, tag="pf", name="ps_warm")
            NWARM = 14
            for n in range(NWARM):
                nc.tensor.matmul(ps_warm, ones_r, ones_r,
                                 start=(n == 0), stop=(n == NWARM - 1))

            loop_ctx = (tc.For_i(0, loop_n, 1) if loop_n
                        else contextlib.nullcontext())
            with loop_ctx:
                # ---- all input DMAs up front: xT (needed first) on both
                # HWDGE queues, then x (needed in phase C) ----
                xts, xpairs = [], []
                for q in range(B_LOC * NPAIR):
                    i0 = 2 * q
                    # the last two pairs ride the ACT queue: their transfers
                    # finish before ACT's first tanh input is even ready, so
                    # they never block ACT compute but double the load cadence
                    eng = nc.scalar if q in (1, 6) else nc.sync
                    xT2 = xtp.tile([P, 2, F], BF16, name="xT2")
                    if q == 0:
                        nc.sync.dma_start(out=xT2[:, 0, :], in_=xt_d.ap()[0])
                        nc.sync.dma_start(out=xT2[:, 1, :], in_=xt_d.ap()[1])
                    else:
                        eng.dma_start(
                            out=xT2,
                            in_=xt_d.ap()[i0:i0 + 2].rearrange("j p f -> p j f"))
                    xts.append(xT2)
                for q in range(B_LOC * NPAIR):
                    i0 = 2 * q
                    x2 = xp.tile([P, 2, F], BF16, name="x2")
                    nc.gpsimd.dma_start(
                        out=x2,
                        in_=x_d.ap()[i0:i0 + 2].rearrange("j p f -> p j f"))
                    xpairs.append(x2)

                w_all_b, rec_b, lws_b, trws_b = [], [], [], []
                cumB_b = [None, None]
                c2_b = [None, None]
                m_all_b = []
                if have_mask:
                    for b in range(B_LOC):
                        m_all = wp.tile([P, NTB], F32, tag="m_all",
                                        name="m_all")
                        nc.sync.dma_start(out=m_all, in_=m_d.ap()[b])
                        m_all_b.append(m_all)

                def emit_phase_a(b):
                    """logits for batch b: per pair matmul->tanh->mul->fold->
                    reduce->exp, with tri/step scales issued per pair."""
                    alpha = wp.tile([P, NTB], BF16, tag="alpha", name="alpha")
                    w_all = wp.tile([P, NTB], F32, tag="w_all", name="w_all")
                    w_all_b.append(w_all)
                    lws, trws = [], []
                    lws_b.append(lws)
                    trws_b.append(trws)
                    for pp in range(NPAIR):
                        ps_v2 = ps_v_pool.tile([P, 2, F], F32, name="ps_v2")
                        for j in range(2):
                            t = NTB * b + 2 * pp + j
                            xT2 = xts[t // 2]
                            for k in range(KC):
                                nc.tensor.matmul(
                                    ps_v2[:, j, :],
                                    xT2[:, t % 2, k * P:(k + 1) * P],
                                    W_sb[:, k, :],
                                    start=(k == 0),
                                    stop=(k == KC - 1 and not have_b),
                                )
                            if have_b:
                                nc.tensor.matmul(ps_v2[:, j, :], ones_row,
                                                 b_sb, start=False, stop=True)
                        v2 = vp.tile([P, 2, F], BF16, name="v2")
                        nc.scalar.activation(out=v2, in_=ps_v2, func=Tanh)
                        scr2 = scrp.tile([P, 2, F], BF16, name="scr2")
                        nc.vector.tensor_mul(scr2, v2, u_bc2)
                        fld = foldp.tile([P, 2, F // 2], BF16, name="fld")
                        nc.vector.tensor_tensor(
                            out=fld, in0=scr2[:, :, 0:F // 2],
                            in1=scr2[:, :, F // 2:F], op=ADD)
                        sl = slice(2 * pp, 2 * pp + 2)
                        with nc.allow_low_precision(
                                reason="bf16 logits; 2e-2 rel-err budget"):
                            nc.vector.tensor_reduce(
                                alpha[:, sl], fld,
                                axis=mybir.AxisListType.X, op=ADD)
                        nc.scalar.activation(out=w_all[:, sl],
                                             in_=alpha[:, sl], func=Exp)
                        if have_mask:
                            nc.vector.tensor_mul(w_all[:, sl], w_all[:, sl],
                                                 m_all_b[b][:, sl])
                        for i in (2 * pp, 2 * pp + 1):
                            if i < NTB - 1:
                                lw = lwp.tile([P, NTB], BF16, name="lw")
                                nc.gpsimd.tensor_scalar_mul(
                                    lw, sm_bf[:, i, :], w_all[:, i:i + 1])
                                lws.append(lw)
                            trw = trwp.tile([P, P], BF16, name="trw")
                            nc.gpsimd.tensor_scalar_mul(
                                trw, tri_bf, w_all[:, i:i + 1])
                            trws.append(trw)

                def emit_zrec(b):
                    # f32r copy on ACT: keeps the congested DVE FIFO clear
                    w_r = wp.tile([P, NTB], F32R, tag="w_r", name="w_r")
                    nc.scalar.activation(out=w_r, in_=w_all_b[b], func=Copy)
                    ps_z = ps_P_pool.tile([P, 2 * NTB], F32, tag="pf",
                                          name="ps_z")
                    nc.tensor.matmul(ps_z[:, 0:NTB], triu_r, w_r,
                                     start=True, stop=True)
                    nc.tensor.matmul(ps_z[:, NTB:2 * NTB], ones_r, w_r,
                                     start=True, stop=True)
                    S = wp.tile([P, NTB], F32, tag="S", name="S")
                    nc.vector.tensor_tensor_scan(
                        out=S, data0=ps_z[:, NTB:2 * NTB], data1=zeros8,
                        initial=0.0, op0=ADD, op1=ADD)
                    Z = wp.tile([P, NTB], F32, tag="Z", name="Z")
                    nc.vector.tensor_tensor(out=Z, in0=S,
                                            in1=ps_z[:, NTB:2 * NTB], op=SUB)
                    nc.vector.tensor_tensor(out=Z, in0=Z, in1=ps_z[:, 0:NTB],
                                            op=ADD)
                    if have_mask:
                        # only a mask can zero the denominator; without one
                        # Z >= exp(alpha) > 0 and the EPS add is dead weight
                        nc.vector.tensor_scalar_add(Z, Z, EPS)
                    rec = wp.tile([P, NTB], F32, tag="rec", name="rec")
                    nc.vector.reciprocal(rec, Z)
                    rec_b.append(rec)

                def emit_offsets(b):
                    # ps_cum[m, f] = sum_{j<m} (w * x) tile totals for batch b
                    ps_cum = ps_P_pool.tile([NTB, F], F32, tag="pf",
                                            name="ps_cum")
                    for j in range(NTB - 1):
                        nc.tensor.matmul(ps_cum, lws_b[b][j],
                                         xpairs[NPAIR * b + j // 2][:, j % 2, :],
                                         start=(j == 0), stop=(j == NTB - 2))
                    cumB = cumbp.tile([NTB, F], BF16, name="cumB")
                    nc.scalar.activation(out=cumB, in_=ps_cum, func=Copy)
                    cumB_b[b] = cumB

                ro_pat = [0, 1, 0, 1, 0, 1, 1, 0, 1, 0, 1, 0, 1, 0, 1, 0]
                ro_n = [0]

                def emit_ctile(b, i):
                    ps_P = ps_P_pool.tile([P, F], F32, tag="pf", name="ps_P")
                    nc.tensor.matmul(ps_P, trws_b[b][i],
                                     xpairs[NPAIR * b + i // 2][:, i % 2, :],
                                     start=True, stop=(i == 0))
                    if i > 0:
                        nc.tensor.matmul(ps_P, sel_bf[:, i * P:(i + 1) * P],
                                         cumB_b[b], start=False, stop=True)
                    rec = rec_b[b]
                    if i % 2 == 0:
                        c2_b[b] = cp.tile([P, 2, F], BF16, name="c2")
                    c_sl = c2_b[b][:, i % 2, :]
                    if ro_pat[ro_n[0]] == 0:
                        nc.scalar.activation(out=c_sl, in_=ps_P, func=Copy,
                                             scale=rec[:, i:i + 1])
                    else:
                        nc.vector.tensor_scalar_mul(c_sl, ps_P,
                                                    rec[:, i:i + 1])
                    ro_n[0] += 1
                    i0 = NTB * b + i
                    if i % 2 == 1:
                        nc.sync.dma_start(
                            out=c_d.ap()[i0 - 1:i0 + 1].rearrange(
                                "j p f -> p j f"),
                            in_=c2_b[b])

                emit_phase_a(0)
                emit_phase_a(1)
                emit_zrec(0)
                emit_offsets(0)
                for i in range(7):
                    emit_ctile(0, i)
                emit_zrec(1)
                emit_offsets(1)
                seq = [(0, 7), (1, 0), (1, 1), (1, 2), (1, 3),
                       (1, 4), (1, 5), (1, 6), (1, 7)]
                for b, i in seq:
                    emit_ctile(b, i)

    nc.compile()
    return nc


_NC_CACHE: dict = {}


def _get_nc(have_b, have_mask, loop_n=0):
    key = (have_b, have_mask, loop_n)
    if key not in _NC_CACHE:
        _NC_CACHE[key] = _build(have_b, have_mask, loop_n)
    return _NC_CACHE[key]


def _host_xt(xs):
    """xs: (NT, P, F) tile-major core shard -> pre-transposed layout where
    xt[i, p, k*128+t] = xs[i, t, k*128+p] (chunk-transposed for matmul lhsT)."""
    v = xs.reshape(NT, P, KC, P).transpose(0, 3, 2, 1)
    return np.ascontiguousarray(v).reshape(NT, P, F)


def make_core_maps(x, W, u, b=None, mask_f=None):
    """Build the 8 per-core input maps from full inputs."""
    x16 = x.astype(NPBF16)
    # W_host[p, k*F + f] = W[k*P + p, f]
    W_r = np.ascontiguousarray(
        W.reshape(KC, P, F).transpose(1, 0, 2).reshape(P, KC * F)).astype(NPBF16)
    u_r = np.ascontiguousarray(u.reshape(1, F)).astype(NPBF16)
    maps = []
    for core in range(NCORES):
        xs = np.ascontiguousarray(
            x16[core * B_LOC:(core + 1) * B_LOC].reshape(NT, P, F))
        m = {"x": xs, "xT": _host_xt(xs), "W": W_r, "u": u_r}
        if b is not None:
            m["b"] = np.ascontiguousarray(b.reshape(1, F)).astype(NPBF16)
        if mask_f is not None:
            ms = mask_f[core * B_LOC:(core + 1) * B_LOC]
            m["m"] = np.ascontiguousarray(
                ms.reshape(B_LOC, NTB, P).transpose(0, 2, 1))
        maps.append(m)
    return maps


def kernel(x, mask, W, b, u):
    x = np.asarray(x, dtype=np.float32)
    W = np.asarray(W, dtype=np.float32)
    b = np.asarray(b, dtype=np.float32)
    u = np.asarray(u, dtype=np.float32)
    mask_f = np.asarray(mask).astype(np.float32)

    have_b = bool(np.any(b != 0.0))
    have_mask = bool(np.any(mask_f != 1.0))

    nc = _get_nc(have_b, have_mask)
    in_maps = make_core_maps(x, W, u,
                             b if have_b else None,
                             mask_f if have_mask else None)
    res = run_bass_kernel_spmd(nc, in_maps, core_ids=list(range(NCORES)))
    out = np.stack([np.asarray(r["c"]).astype(np.float32).reshape(B_LOC, T, F)
                    for r in res.results])
    return out.reshape(B, T, F)


# revision 73
# speedup vs baseline: 2.8126x; 1.0299x over previous
"""Trainium2 Bass kernel for nn_Attention (cumulative masked softmax attention).

Reference computation:
    v   = tanh(x @ W + b)                  (B, T, F)
    a   = v . u                            (B, T)   -- query-independent logits
    e   = exp(a)[:, None, :] * tril * mask (B, T, T)
    alf = e / (sum_s e + EPS)
    c   = alf @ x                          (B, T, F)

Because the logits are query-independent and the mask is lower-triangular,
the (B,T,T) softmax-matmul collapses to a running weighted average:
    w[s]  = exp(a[s]) * mask[s]
    c[t]  = cumsum_s(w * x)[t] / (cumsum_s(w)[t] + EPS)
which is O(B*T*F) instead of O(B*T^2*F).

Sharding: data-parallel over batch B across 8 NeuronCores (2 batches/core).

v3 design (bf16):
  - All HBM traffic in bf16 (x, xT, W, u, c out) -- rel-err budget is 2e-2,
    bf16 keeps it ~2e-3 while halving DMA bytes.
  - The weights w fold into the 128x128 triangular matrix (tri_w = tri * w
    per tile, a cheap [128,128] gpsimd scale) instead of scaling x.
  - Cross-tile prefix offsets via 7 "step-mask" matmuls per batch (lhsT
    column m gets w[:,j] iff m > j) writing the per-tile offset rows [8, F]
    in PSUM; one copy to SBUF, then one [8,128]-selector matmul per tile
    broadcasts its offset row onto the tile prefix.
  - Denominator Z = prefix(w) via two tiny [P,8] matmuls + a free-dim scan
    (f32), reciprocal folded into the PSUM->SBUF readout scale.
  - Scheduling: xT loads split across the SP and ACT DMA queues ahead of
    the x loads; exp/tri-scales issued per pair so phase C of batch 0 can
    fill the tensor engine while batch 1's logits chain completes.
"""

import contextlib

import numpy as np
import ml_dtypes

import concourse.bass as bass  # noqa: F401
import concourse.tile as tile
from concourse import bacc, mybir
from concourse.bass_utils import run_bass_kernel_spmd

B, T, F = 16, 1024, 512
EPS = 1e-7
NCORES = 8
B_LOC = B // NCORES          # batches per core
R = B_LOC * T                # rows per core
P = 128                      # partition tile
NT = R // P                  # row tiles per core
NTB = T // P                 # row tiles per batch
NPAIR = NTB // 2             # tile pairs per batch
KC = F // P                  # contraction chunks

F32 = mybir.dt.float32
F32R = mybir.dt.float32r
BF16 = mybir.dt.bfloat16
NPBF16 = ml_dtypes.bfloat16


def _build(have_b: bool, have_mask: bool, loop_n: int = 0):
    """Build the per-core Bass module. loop_n > 0 wraps the body in a
    hardware For_i loop (used only for timing)."""
    nc = bacc.Bacc("TRN2", target_bir_lowering=False, debug=False)

    x_d = nc.dram_tensor("x", [NT, P, F], BF16, kind="ExternalInput")
    xt_d = nc.dram_tensor("xT", [NT, P, F], BF16, kind="ExternalInput")
    # W pre-arranged on host as (P, KC*F): W_host[p, k*F+f] = W[k*P+p, f]
    w_d = nc.dram_tensor("W", [P, KC * F], BF16, kind="ExternalInput")
    u_d = nc.dram_tensor("u", [1, F], BF16, kind="ExternalInput")
    if have_b:
        b_d = nc.dram_tensor("b", [1, F], BF16, kind="ExternalInput")
    if have_mask:
        m_d = nc.dram_tensor("m", [B_LOC, P, NTB], F32, kind="ExternalInput")
    c_d = nc.dram_tensor("c", [NT, P, F], BF16, kind="ExternalOutput")

    Tanh = mybir.ActivationFunctionType.Tanh
    Exp = mybir.ActivationFunctionType.Exp
    Copy = mybir.ActivationFunctionType.Copy
    ADD = mybir.AluOpType.add
    SUB = mybir.AluOpType.subtract

    with tile.TileContext(nc) as tc:
        with (
            tc.tile_pool(name="const", bufs=1) as const,
            tc.tile_pool(name="xp", bufs=2 * NPAIR) as xp,
            tc.tile_pool(name="xtp", bufs=2 * NPAIR) as xtp,
            tc.tile_pool(name="vp", bufs=2) as vp,
            tc.tile_pool(name="scrp", bufs=2) as scrp,
            tc.tile_pool(name="foldp", bufs=2) as foldp,
            tc.tile_pool(name="wp", bufs=2) as wp,
            tc.tile_pool(name="trwp", bufs=10) as trwp,
            tc.tile_pool(name="lwp", bufs=8) as lwp,
            tc.tile_pool(name="cumbp", bufs=2) as cumbp,
            tc.tile_pool(name="cp", bufs=8) as cp,
            tc.tile_pool(name="ps_v", bufs=2, space="PSUM") as ps_v_pool,
            tc.tile_pool(name="ps_P", bufs=4, space="PSUM") as ps_P_pool,
        ):
            # ---- constants ----
            # W split into per-chunk DMAs on the gpsimd queue so the first
            # matmul only waits on chunk 0 (+ the first xT tile) and the SP
            # queue can issue the xT loads immediately.
            W_sb = const.tile([P, KC, F], BF16)
            wr_ap = w_d.ap().rearrange("p (k f) -> p k f", k=KC)
            nc.scalar.dma_start(out=W_sb, in_=wr_ap)
            u_bc2 = const.tile([P, 2, F], BF16)
            nc.gpsimd.dma_start(out=u_bc2[:, 0, :],
                                in_=u_d.ap().to_broadcast((P, F)))
            nc.gpsimd.dma_start(out=u_bc2[:, 1, :],
                                in_=u_d.ap().to_broadcast((P, F)))
            onesf = const.tile([P, P], F32)
            nc.vector.memset(onesf, 1.0)
            ones_r = const.tile([P, P], F32R)
            nc.vector.tensor_copy(ones_r, onesf)

            if have_b:
                b_sb = const.tile([1, F], BF16)
                nc.sync.dma_start(out=b_sb, in_=b_d.ap())
                ones_row = const.tile([1, P], BF16)
                nc.vector.memset(ones_row, 1.0)

            # triangular matrices: triu[p, m] = 1 iff p <= m (inclusive prefix)
            triu_f = const.tile([P, P], F32)
            nc.gpsimd.memset(triu_f, 0.0)
            nc.gpsimd.affine_select(
                out=triu_f, in_=triu_f, compare_op=mybir.AluOpType.is_gt,
                fill=1.0, base=0, pattern=[[-1, P]], channel_multiplier=1)
            tri_bf = const.tile([P, P], BF16)
            nc.vector.tensor_copy(tri_bf, triu_f)
            triu_r = const.tile([P, P], F32R)
            nc.vector.tensor_copy(triu_r, triu_f)
            zeros8 = const.tile([P, NTB], F32)
            nc.vector.memset(zeros8, 0.0)
            # step masks: sm[j][p, m] = 1 iff m > j (offset matmul lhsT)
            sm_f = const.tile([P, NTB - 1, NTB], F32)
            sm_bf = const.tile([P, NTB - 1, NTB], BF16)
            nc.gpsimd.memset(sm_f, 1.0)
            for j in range(NTB - 1):
                nc.gpsimd.affine_select(
                    out=sm_f[:, j, :], in_=sm_f[:, j, :],
                    compare_op=mybir.AluOpType.is_gt,
                    fill=0.0, base=-j, pattern=[[1, NTB]], channel_multiplier=0)
            nc.vector.tensor_copy(sm_bf, sm_f)
            # row selectors: sel8[p, i*P+m] = 1 iff p == i (broadcast matmuls)
            sel_f = const.tile([NTB, NTB * P], F32)
            sel_bf = const.tile([NTB, NTB * P], BF16)
            nc.gpsimd.memset(sel_f, 1.0)
            nc.gpsimd.affine_select(
                out=sel_f, in_=sel_f, compare_op=mybir.AluOpType.is_equal,
                fill=0.0, base=0, pattern=[[-1, NTB], [0, P]],
                channel_multiplier=1)
            nc.vector.tensor_copy(sel_bf, sel_f)

            # ramp the PE clock gate while the first DMAs are in flight
            ps_warm = ps_P_pool.tile([P, P], F32, tag="pf", name="ps_warm")
            NWARM = 14
            for n in range(NWARM):
                nc.tensor.matmul(ps_warm, ones_r, ones_r,
                                 start=(n == 0), stop=(n == NWARM - 1))

            loop_ctx = (tc.For_i(0, loop_n, 1) if loop_n
                        else contextlib.nullcontext())
            with loop_ctx:
                # ---- all input DMAs up front: xT (needed first) on both
                # HWDGE queues, then x (needed in phase C) ----
                xts, xpairs = [], []
                for q in range(B_LOC * NPAIR):
                    i0 = 2 * q
                    # the last two pairs ride the ACT queue: their transfers
                    # finish before ACT's first tanh input is even ready, so
                    # they never block ACT compute but double the load cadence
                    eng = nc.scalar if q in (1, 6) else nc.sync
                    xT2 = xtp.tile([P, 2, F], BF16, name="xT2")
                    if q == 0:
                        nc.sync.dma_start(out=xT2[:, 0, :], in_=xt_d.ap()[0])
                        nc.sync.dma_start(out=xT2[:, 1, :], in_=xt_d.ap()[1])
                    else:
                        eng.dma_start(
                            out=xT2,
                            in_=xt_d.ap()[i0:i0 + 2].rearrange("j p f -> p j f"))
                    xts.append(xT2)
                for q in range(B_LOC * NPAIR):
                    i0 = 2 * q
                    x2 = xp.tile([P, 2, F], BF16, name="x2")
                    nc.gpsimd.dma_start(
                        out=x2,
                        in_=x_d.ap()[i0:i0 + 2].rearrange("j p f -> p j f"))
                    xpairs.append(x2)

                w_all_b, rec_b, lws_b, trws_b = [], [], [], []
                cumB_b = [None, None]
                c2_b = [None, None]
                m_all_b = []
                if have_mask:
                    for b in range(B_LOC):
                        m_all = wp.tile([P, NTB], F32, tag="m_all",
                                        name="m_all")
                        nc.sync.dma_start(out=m_all, in_=m_d.ap()[b])
                        m_all_b.append(m_all)

                def emit_phase_a(b):
                    """logits for batch b: per pair matmul->tanh->mul->fold->
                    reduce->exp, with tri/step scales issued per pair."""
                    alpha = wp.tile([P, NTB], BF16, tag="alpha", name="alpha")
                    w_all = wp.tile([P, NTB], F32, tag="w_all", name="w_all")
                    w_all_b.append(w_all)
                    lws, trws = [], []
                    lws_b.append(lws)
                    trws_b.append(trws)
                    for pp in range(NPAIR):
                        ps_v2 = ps_v_pool.tile([P, 2, F], F32, name="ps_v2")
                        for j in range(2):
                            t = NTB * b + 2 * pp + j
                            xT2 = xts[t // 2]
                            for k in range(KC):
                                nc.tensor.matmul(
                                    ps_v2[:, j, :],
                                    xT2[:, t % 2, k * P:(k + 1) * P],
                                    W_sb[:, k, :],
                                    start=(k == 0),
                                    stop=(k == KC - 1 and not have_b),
                                )
                            if have_b:
                                nc.tensor.matmul(ps_v2[:, j, :], ones_row,
                                                 b_sb, start=False, stop=True)
                        v2 = vp.tile([P, 2, F], BF16, name="v2")
                        nc.scalar.activation(out=v2, in_=ps_v2, func=Tanh)
                        scr2 = scrp.tile([P, 2, F], BF16, name="scr2")
                        nc.vector.tensor_mul(scr2, v2, u_bc2)
                        fld = foldp.tile([P, 2, F // 2], BF16, name="fld")
                        nc.vector.tensor_tensor(
                            out=fld, in0=scr2[:, :, 0:F // 2],
                            in1=scr2[:, :, F // 2:F], op=ADD)
                        sl = slice(2 * pp, 2 * pp + 2)
                        with nc.allow_low_precision(
                                reason="bf16 logits; 2e-2 rel-err budget"):
                            nc.vector.tensor_reduce(
                                alpha[:, sl], fld,
                                axis=mybir.AxisListType.X, op=ADD)
                        nc.scalar.activation(out=w_all[:, sl],
                                             in_=alpha[:, sl], func=Exp)
                        if have_mask:
                            nc.vector.tensor_mul(w_all[:, sl], w_all[:, sl],
                                                 m_all_b[b][:, sl])
                        for i in (2 * pp, 2 * pp + 1):
                            if i < NTB - 1:
                                lw = lwp.tile([P, NTB], BF16, name="lw")
                                nc.gpsimd.tensor_scalar_mul(
                                    lw, sm_bf[:, i, :], w_all[:, i:i + 1])
                                lws.append(lw)
                            trw = trwp.tile([P, P], BF16, name="trw")
                            nc.gpsimd.tensor_scalar_mul(
                                trw, tri_bf, w_all[:, i:i + 1])
                            trws.append(trw)

                def emit_zrec(b):
                    # f32r copy on ACT: keeps the congested DVE FIFO clear
                    w_r = wp.tile([P, NTB], F32R, tag="w_r", name="w_r")
                    nc.scalar.activation(out=w_r, in_=w_all_b[b], func=Copy)
                    ps_z = ps_P_pool.tile([P, 2 * NTB], F32, tag="pf",
                                          name="ps_z")
                    nc.tensor.matmul(ps_z[:, 0:NTB], triu_r, w_r,
                                     start=True, stop=True)
                    nc.tensor.matmul(ps_z[:, NTB:2 * NTB], ones_r, w_r,
                                     start=True, stop=True)
                    S = wp.tile([P, NTB], F32, tag="S", name="S")
                    nc.vector.tensor_tensor_scan(
                        out=S, data0=ps_z[:, NTB:2 * NTB], data1=zeros8,
                        initial=0.0, op0=ADD, op1=ADD)
                    Z = wp.tile([P, NTB], F32, tag="Z", name="Z")
                    nc.vector.tensor_tensor(out=Z, in0=S,
                                            in1=ps_z[:, NTB:2 * NTB], op=SUB)
                    nc.vector.tensor_tensor(out=Z, in0=Z, in1=ps_z[:, 0:NTB],
                                            op=ADD)
                    if have_mask:
                        # only a mask can zero the denominator; without one
                        # Z >= exp(alpha) > 0 and the EPS add is dead weight
                        nc.vector.tensor_scalar_add(Z, Z, EPS)
                    rec = wp.tile([P, NTB], F32, tag="rec", name="rec")
                    nc.vector.reciprocal(rec, Z)
                    rec_b.append(rec)

                def emit_offsets(b):
                    # ps_cum[m, f] = sum_{j<m} (w * x) tile totals for batch b
                    ps_cum = ps_P_pool.tile([NTB, F], F32, tag="pf",
                                            name="ps_cum")
                    for j in range(NTB - 1):
                        nc.tensor.matmul(ps_cum, lws_b[b][j],
                                         xpairs[NPAIR * b + j // 2][:, j % 2, :],
                                         start=(j == 0), stop=(j == NTB - 2))
                    cumB = cumbp.tile([NTB, F], BF16, name="cumB")
                    nc.scalar.activation(out=cumB, in_=ps_cum, func=Copy)
                    cumB_b[b] = cumB

                ro_pat = [0, 1, 0, 1, 0, 1, 1, 0, 1, 0, 1, 0, 1, 0, 1, 0]
                ro_n = [0]

                def emit_ctile(b, i):
                    ps_P = ps_P_pool.tile([P, F], F32, tag="pf", name="ps_P")
                    nc.tensor.matmul(ps_P, trws_b[b][i],
                                     xpairs[NPAIR * b + i // 2][:, i % 2, :],
                                     start=True, stop=(i == 0))
                    if i > 0:
                        nc.tensor.matmul(ps_P, sel_bf[:, i * P:(i + 1) * P],
                                         cumB_b[b], start=False, stop=True)
                    rec = rec_b[b]
                    if i % 2 == 0:
                        c2_b[b] = cp.tile([P, 2, F], BF16, name="c2")
                    c_sl = c2_b[b][:, i % 2, :]
                    if ro_pat[ro_n[0]] == 0:
                        nc.scalar.activation(out=c_sl, in_=ps_P, func=Copy,
                                             scale=rec[:, i:i + 1])
                    else:
                        nc.vector.tensor_scalar_mul(c_sl, ps_P,
                                                    rec[:, i:i + 1])
                    ro_n[0] += 1
                    i0 = NTB * b + i
                    if i % 2 == 1:
                        nc.sync.dma_start(
                            out=c_d.ap()[i0 - 1:i0 + 1].rearrange(
                                "j p f -> p j f"),
                            in_=c2_b[b])

                emit_phase_a(0)
                emit_phase_a(1)
                emit_zrec(0)
                emit_offsets(0)
                for i in range(7):
                    emit_ctile(0, i)
                emit_zrec(1)
                emit_offsets(1)
                seq = [(0, 7), (1, 0), (1, 1), (1, 2), (1, 3),
                       (1, 4), (1, 5), (1, 6), (1, 7)]
                for b, i in seq:
                    emit_ctile(b, i)

    nc.compile()
    return nc


_NC_CACHE: dict = {}


def _get_nc(have_b, have_mask, loop_n=0):
    key = (have_b, have_mask, loop_n)
    if key not in _NC_CACHE:
        _NC_CACHE[key] = _build(have_b, have_mask, loop_n)
    return _NC_CACHE[key]


def _host_xt(xs):
    """xs: (NT, P, F) tile-major core shard -> pre-transposed layout where
    xt[i, p, k*128+t] = xs[i, t, k*128+p] (chunk-transposed for matmul lhsT)."""
    v = xs.reshape(NT, P, KC, P).transpose(0, 3, 2, 1)
    return np.ascontiguousarray(v).reshape(NT, P, F)


def make_core_maps(x, W, u, b=None, mask_f=None):
    """Build the 8 per-core input maps from full inputs."""
    x16 = x.astype(NPBF16)
    # W_host[p, k*F + f] = W[k*P + p, f]
    W_r = np.ascontiguousarray(
        W.reshape(KC, P, F).transpose(1, 0, 2).reshape(P, KC * F)).astype(NPBF16)
    u_r = np.ascontiguousarray(u.reshape(1, F)).astype(NPBF16)
    maps = []
    for core in range(NCORES):
        xs = np.ascontiguousarray(
            x16[core * B_LOC:(core + 1) * B_LOC].reshape(NT, P, F))
        m = {"x": xs, "xT": _host_xt(xs), "W": W_r, "u": u_r}
        if b is not None:
            m["b"] = np.ascontiguousarray(b.reshape(1, F)).astype(NPBF16)
        if mask_f is not None:
            ms = mask_f[core * B_LOC:(core + 1) * B_LOC]
            m["m"] = np.ascontiguousarray(
                ms.reshape(B_LOC, NTB, P).transpose(0, 2, 1))
        maps.append(m)
    return maps


def kernel(x, mask, W, b, u):
    x = np.asarray(x, dtype=np.float32)
    W = np.asarray(W, dtype=np.float32)
    b = np.asarray(b, dtype=np.float32)
    u = np.asarray(u, dtype=np.float32)
    mask_f = np.asarray(mask).astype(np.float32)

    have_b = bool(np.any(b != 0.0))
    have_mask = bool(np.any(mask_f != 1.0))

    nc = _get_nc(have_b, have_mask)
    in_maps = make_core_maps(x, W, u,
                             b if have_b else None,
                             mask_f if have_mask else None)
    res = run_bass_kernel_spmd(nc, in_maps, core_ids=list(range(NCORES)))
    out = np.stack([np.asarray(r["c"]).astype(np.float32).reshape(B_LOC, T, F)
                    for r in res.results])
    return out.reshape(B, T, F)
